# revision 1
# baseline (speedup 1.0000x reference)
"""Multi-head self-attention TRN2 Bass kernel.

Problem: x[2, 2048, 1024], 16 heads x 64 dim, fp32.
Sharding: 8 cores = 2 batches x 4 head-groups (4 heads each).
Each core computes its batch's partial output (its 4 heads through
QKV -> attention -> output projection rows); host sums the 4 partials
per batch and adds bo.

Per-core structure (avoids every attention transpose):
  - x^T pre-tiled on host (bf16) and loaded as 4 contiguous 1MB DMAs,
    chained so chunk 0 lands first at full bandwidth.
  - q^T, k^T [256, 2048] bf16  (head h at partitions (h%2)*64 of tile h//2)
  - V' [2048, 4, 65] bf16  (per head: V columns + a ones column)
  - scores computed TRANSPOSED: S^T[k,q] = k^T.T @ q^T as row-tiled
    head-PAIRS (two concurrent K=64 matmuls); 1/sqrt(hd) folded into
    Wq/bq on host.
  - exp on ACT -> A^T bf16 (rolling 4-deep buffer), directly the moving
    operand of out^T[65, q] = V'^T @ A^T; row 64 = softmax row sums
    (ones-column trick).
  - normalize: fast-reciprocal on DVE; the [1,512] -> [64,512] partition
    broadcast is a rank-1 PE outer product (ones[1,64].T @ recip).
  - out_proj: head pairs stacked to K=128, emitted two q-tiles per pair
    boundary so it never stalls the in-order PE stream.
  - the ACT exp stream is the bottleneck (~139us busy); the pre-exp
    critical path carries only kT m=0 + V + qT chunk 0, while kT m=1 and
    qT chunks 1-3 drain through the attention loop's slack slots. Dummy
    matmuls fill residual PE bubbles to keep the HAM clock-gate warm at
    2.4 GHz (cold phases run the PE at 1.2 GHz).
"""

import numpy as np

S = 2048          # sequence length per batch
H = 1024          # hidden
G = 256           # head-group width (4 heads x 64)
HD = 65           # V' columns per head (64 + ones)
NHL = 4           # heads per core
N_CORES = 8

_CACHE = {}


def _build():
    if "nc" in _CACHE:
        return _CACHE["nc"]

    import concourse.bass as bass
    import concourse.mybir as mybir
    import concourse.tile as tile
    from concourse import bacc
    from concourse.tile_rust import add_dep_helper

    f32 = mybir.dt.float32
    bf16 = mybir.dt.bfloat16
    EXP = mybir.ActivationFunctionType.Exp

    nc = bacc.Bacc("TRN2", target_bir_lowering=False, debug=False,
                   num_devices=N_CORES)

    xt_in = nc.dram_tensor("xt", [4, 128, 8, 512], bf16, kind="ExternalInput")
    wq_in = nc.dram_tensor("wq", [H, G], bf16, kind="ExternalInput")
    wk_in = nc.dram_tensor("wk", [H, G], bf16, kind="ExternalInput")
    wv_in = nc.dram_tensor("wv", [H, G], bf16, kind="ExternalInput")
    bq_in = nc.dram_tensor("bq", [G, 1], f32, kind="ExternalInput")
    bk_in = nc.dram_tensor("bk", [G, 1], f32, kind="ExternalInput")
    bv_in = nc.dram_tensor("bv", [G], f32, kind="ExternalInput")
    wo_in = nc.dram_tensor("wo", [NHL, 64, H], bf16, kind="ExternalInput")
    out_d = nc.dram_tensor("out", [S, H], f32, kind="ExternalOutput")

    with tile.TileContext(nc) as tc:
        with tc.tile_pool(name="persist", bufs=1) as persist:
            qT = persist.tile([128, 2, S], bf16)     # [qd, m, s]
            kT = persist.tile([128, 2, S], bf16)
            vp = persist.tile([128, 16, NHL, HD], bf16)  # [s-part, st, h, col]
            bq_sb = persist.tile([128, 2, 1], f32)
            bk_sb = persist.tile([128, 2, 1], f32)
            bv_bc = persist.tile([128, G], f32)
            wo_pr = persist.tile([128, 2, H], bf16)
            ones64 = persist.tile([1, 64], bf16)

            # ------- Phase A + q-chunk-0 scores/exp overlap -------
            w_pool = persist
            xT_pool = persist
            with (
                tc.tile_pool(name="ps_a", bufs=2, space="PSUM") as ps_a,
            ):
                wq_sb = w_pool.tile([128, 8, G], bf16)
                wk_sb = w_pool.tile([128, 8, G], bf16)
                wv_sb = w_pool.tile([128, 8, G], bf16)
                nc.sync.dma_start(
                    out=wq_sb, in_=wq_in.ap().rearrange("(t p) d -> p t d", p=128))
                nc.sync.dma_start(
                    out=wk_sb, in_=wk_in.ap().rearrange("(t p) d -> p t d", p=128))
                nc.sync.dma_start(
                    out=wv_sb, in_=wv_in.ap().rearrange("(t p) d -> p t d", p=128))

                # per-chunk x^T tiles; host pre-tiles x^T into exactly
                # this layout so each chunk is one contiguous 1MB DMA
                xTc = [xT_pool.tile([128, 8, 512], bf16, name=f"xT_{jc}")
                       for jc in range(4)]
                x_dmas = [nc.sync.dma_start(out=xTc[jc], in_=xt_in.ap()[jc])
                          for jc in range(4)]
                for jc in range(1, 4):
                    # chain the chunk loads so chunk 0 lands first at full
                    # bandwidth instead of round-robin across all four
                    add_dep_helper(x_dmas[jc].ins, x_dmas[jc - 1].ins,
                                   reason="serialize x chunk loads")

                nc.sync.dma_start(
                    out=bq_sb, in_=bq_in.ap().rearrange("(m p) o -> p m o", p=128))
                nc.sync.dma_start(
                    out=bk_sb, in_=bk_in.ap().rearrange("(m p) o -> p m o", p=128))
                # broadcast bv along partitions (stride-0 partition AP)
                bv_ap = bass.AP(tensor=bv_in, offset=0, ap=[[0, 128], [1, G]])
                nc.gpsimd.dma_start(out=bv_bc, in_=bv_ap)
                # Wo as stacked head pairs: [two*64+p, pr, n]
                nc.sync.dma_start(
                    out=wo_pr,
                    in_=wo_in.ap().rearrange("(pr two) p n -> (two p) pr n", two=2))
                # ones columns of V'
                nc.gpsimd.memset(vp[:, :, :, 64:65], 1.0)
                nc.gpsimd.memset(ones64, 1.0)

                def dummy_a(n=512):
                    ps_d = ps_a.tile([128, 512], f32, tag="dum", bufs=1)
                    nc.tensor.matmul(
                        ps_d[:, 0:n], lhsT=wq_sb[:, 0, 0:128],
                        rhs=wq_sb[:, 0:2, :].rearrange("p a b -> p (a b)")[:, 0:n],
                        start=True, stop=True)

                def qk_half(w_sb, b_sb, dst, jc, m, half, st):
                    sl = slice(jc * 512, (jc + 1) * 512)
                    if half == 0:
                        st["ps"] = ps_a.tile([128, 512], f32, tag="qk",
                                             name=f"psq_{id(w_sb)}_{jc}_{m}")
                    for ht in range(half * 4, half * 4 + 4):
                        mm = nc.tensor.matmul(
                            st["ps"],
                            lhsT=w_sb[:, ht, m * 128:(m + 1) * 128],
                            rhs=xTc[jc][:, ht, :],
                            start=(ht == 0), stop=(ht == 7))
                        st.setdefault("first_mm", mm)
                    if half == 1:
                        nc.vector.tensor_scalar_add(
                            dst[:, m, sl], st["ps"], b_sb[:, m, :])

                def v_unit(st16):
                    ps_vt = ps_a.tile([128, 512], f32, tag="qk",
                                      name=f"psv_{st16}")
                    for ht in range(8):
                        nc.tensor.matmul(
                            ps_vt[:, 0:G],
                            lhsT=xTc[st16 // 4][:, ht,
                                                (st16 % 4) * 128:
                                                (st16 % 4 + 1) * 128],
                            rhs=wv_sb[:, ht, :],
                            start=(ht == 0), stop=(ht == 7))
                    nc.vector.tensor_add(
                        vp[:, st16, :, 0:64],
                        ps_vt[:, 0:G].rearrange("p (h d) -> p h d", h=NHL),
                        bv_bc.rearrange("p (h d) -> p h d", h=NHL))

                # minimal pre-exp critical path: kT m=0 + V (needed by the
                # attn@V interleave) + qT chunk 0. kT m=1 and qT chunks 1-3
                # are deferred into the attention stream's slack slots.
                for _ in range(10):
                    dummy_a()
                for jc in range(4):
                    st = {}
                    qk_half(wk_sb, bk_sb, kT, jc, 0, 0, st)
                    qk_half(wk_sb, bk_sb, kT, jc, 0, 1, st)
                    for i in range(4):
                        v_unit(jc * 4 + i)
                for m in range(2):
                    st = {}
                    qk_half(wq_sb, bq_sb, qT, 0, m, 0, st)
                    qk_half(wq_sb, bq_sb, qT, 0, m, 1, st)
            # ---------------- Phase B: attention + out_proj ----------------
            with (
                tc.tile_pool(name="at_roll", bufs=2) as at_pool,
                tc.tile_pool(name="outP", bufs=4) as op_pool,
                tc.tile_pool(name="tmpo", bufs=1) as tmpo_pool,
                tc.tile_pool(name="sums", bufs=4) as sums_pool,
                tc.tile_pool(name="osb", bufs=2) as osb_pool,
                tc.tile_pool(name="ps_s", bufs=2, space="PSUM") as ps_s_pool,
                tc.tile_pool(name="ps_av", bufs=2, space="PSUM") as ps_av_pool,
                tc.tile_pool(name="ps_op", bufs=1, space="PSUM") as ps_op_pool,
            ):
                def dummy(n):
                    ps_d = ps_op_pool.tile([128, 512], f32, tag="dummy")
                    nc.tensor.matmul(ps_d[:, 0:n], lhsT=kT[:, 0, 0:128],
                                     rhs=qT[:, 0, 0:n], start=True, stop=True)

                def norm_head(outP, ps_av, hh, qc, mt):
                    # evacuate PSUM right away to release the bank; run the
                    # normalize chain from SBUF
                    uout = tmpo_pool.tile([HD, 512], f32, tag="uout",
                                          name=f"uo_{qc}_{mt}_{hh}", bufs=4)
                    nc.vector.tensor_copy(uout, ps_av)
                    sums = sums_pool.tile([1, 512], f32, tag="sums",
                                          name=f"sm_{qc}_{mt}_{hh}")
                    nc.vector.tensor_copy(sums, uout[64:65, :])
                    recip = sums_pool.tile([1, 512], f32, tag="recip",
                                           name=f"rc_{qc}_{mt}_{hh}")
                    nc.vector.reciprocal_approx_fast(out=recip, in_=sums)
                    recip_bf = sums_pool.tile([1, 512], bf16, tag="recipb",
                                              name=f"rcb_{qc}_{mt}_{hh}")
                    nc.vector.tensor_copy(recip_bf, recip)
                    # broadcast along partitions: rank-1 outer product on
                    # the PE (ones[1,64].T @ recip[1,512] -> [64,512])
                    rbc = ps_op_pool.tile([64, 512], f32, tag="dummy",
                                          name=f"rb_{qc}_{mt}_{hh}")
                    nc.tensor.matmul(rbc, lhsT=ones64, rhs=recip_bf,
                                     start=True, stop=True)
                    nc.vector.tensor_mul(
                        outP[hh * 64:hh * 64 + 64, :], uout[0:64, :], rbc)

                fillers = []

                def fill_qk_half(w_sb, b_sb, dst, jc, m, half, st):
                    sl = slice(jc * 512, (jc + 1) * 512)
                    if half == 0:
                        st["ps"] = ps_op_pool.tile(
                            [128, 512], f32, tag="dummy",
                            name=f"psf_{id(w_sb)}_{jc}_{m}")
                    for ht in range(half * 4, half * 4 + 4):
                        nc.tensor.matmul(
                            st["ps"],
                            lhsT=w_sb[:, ht, m * 128:(m + 1) * 128],
                            rhs=xTc[jc][:, ht, :],
                            start=(ht == 0), stop=(ht == 7))
                    if half == 1:
                        nc.vector.tensor_scalar_add(
                            dst[:, m, sl], st["ps"], b_sb[:, m, :])

                def add_fill(w_sb, b_sb, dst, jc, m):
                    st = {}
                    fillers.append(lambda: fill_qk_half(
                        w_sb, b_sb, dst, jc, m, 0, st))
                    fillers.append(lambda: fill_qk_half(
                        w_sb, b_sb, dst, jc, m, 1, st))

                for jc in range(4):
                    add_fill(wk_sb, bk_sb, kT, jc, 1)
                for jc in range(1, 4):
                    for m in range(2):
                        add_fill(wq_sb, bq_sb, qT, jc, m)

                def oproj_unit(qc, outPs, qt, tail=False):
                    # out_proj for one q-tile (K=128 stacked pairs); at the
                    # kernel tail the freed score slots double-buffer it
                    osb = osb_pool.tile([128, H], f32, tag="osb",
                                        name=f"osb_{qc}_{qt}")
                    for ncx in range(2):
                        if tail:
                            ps_op = ps_s_pool.tile(
                                [128, 2, 512], f32, tag="s",
                                name=f"psot_{qc}_{qt}_{ncx}")[:, 0, :]
                        else:
                            ps_op = ps_op_pool.tile(
                                [128, 512], f32, tag="oproj",
                                name=f"pso_{qc}_{qt}_{ncx}")
                        for pr in range(2):
                            nc.tensor.matmul(
                                ps_op,
                                lhsT=outPs[pr][:, qt * 128:(qt + 1) * 128],
                                rhs=wo_pr[:, pr, ncx * 512:(ncx + 1) * 512],
                                start=(pr == 0), stop=(pr == 1))
                        nc.vector.tensor_copy(
                            osb[:, ncx * 512:(ncx + 1) * 512], ps_op)
                    nc.sync.dma_start(
                        out=out_d.ap()[qc * 512 + qt * 128:
                                       qc * 512 + (qt + 1) * 128, :],
                        in_=osb)

                prev = None  # (qc, outPs, next_qt) awaiting out_proj
                for qc in range(4):  # q-chunks of 512
                    qsl = slice(qc * 512, (qc + 1) * 512)
                    outPs = []
                    for mt in range(2):  # head pair (2mt, 2mt+1)
                        attnT = at_pool.tile([128, 2, 4, 512], bf16,
                                             tag="at", name=f"at_{qc}_{mt}")
                        ps_avs = [ps_av_pool.tile([HD, 512], f32, tag="av",
                                                  name=f"av_{qc}_{mt}_{hh}")
                                  for hh in range(2)]
                        for kt in range(16):
                            ps_s = ps_s_pool.tile([128, 2, 512], f32, tag="s")
                            for hh in range(2):
                                nc.tensor.matmul(
                                    ps_s[:, hh, :],
                                    lhsT=kT[hh * 64:hh * 64 + 64, mt,
                                            kt * 128:(kt + 1) * 128],
                                    rhs=qT[hh * 64:hh * 64 + 64, mt, qsl],
                                    start=True, stop=True)
                            nc.scalar.activation(
                                out=attnT[:, :, kt % 4, :], in_=ps_s, func=EXP)
                            for hh in range(2):
                                nc.tensor.matmul(
                                    ps_avs[hh],
                                    lhsT=vp[:, kt, 2 * mt + hh, :],
                                    rhs=attnT[:, hh, kt % 4, :],
                                    start=(kt == 0), stop=(kt == 15))
                            if fillers and kt % 2 == 1:
                                fillers.pop(0)()
                            elif kt % 4 == 0:
                                dummy(256)
                        outP = op_pool.tile([128, 512], bf16, tag="outP",
                                            name=f"outP_{qc}_{mt}")
                        for hh in range(2):
                            norm_head(outP, ps_avs[hh], hh, qc, mt)
                        outPs.append(outP)
                        for _ in range(2):
                            dummy(512)
                        if prev is not None:
                            # two q-tiles of the previous q-chunk's out_proj
                            # at each pair boundary
                            pq, pouts, qt0 = prev
                            oproj_unit(pq, pouts, qt0)
                            oproj_unit(pq, pouts, qt0 + 1)
                            prev = (pq, pouts, qt0 + 2) if qt0 + 2 < 4 else None
                    prev = (qc, outPs, 0)
                # cover the last normalize chain, then final out_proj
                for _ in range(4):
                    dummy(512)
                pq, pouts, qt0 = prev
                for qt in range(qt0, 4):
                    oproj_unit(pq, pouts, qt, tail=True)

    nc.compile()
    _CACHE["nc"] = nc
    return nc


def make_in_maps(x, Wq, bq, Wk, bk, Wv, bv, Wo):
    import ml_dtypes
    bf = ml_dtypes.bfloat16

    x = np.asarray(x, dtype=np.float32)
    Wq = np.asarray(Wq, dtype=np.float32)
    bq = np.asarray(bq, dtype=np.float32)
    Wk = np.asarray(Wk, dtype=np.float32)
    bk = np.asarray(bk, dtype=np.float32)
    Wv = np.asarray(Wv, dtype=np.float32)
    bv = np.asarray(bv, dtype=np.float32)
    Wo = np.asarray(Wo, dtype=np.float32)

    scale = np.float32(1.0 / 8.0)  # 1/sqrt(64)

    in_maps = []
    for core in range(N_CORES):
        b = core // 4
        g = core % 4
        cs = slice(g * G, (g + 1) * G)
        in_maps.append({
            "xt": np.ascontiguousarray(
                x[b].reshape(4, 512, 8, 128).transpose(0, 3, 2, 1)).astype(bf),
            "wq": np.ascontiguousarray(Wq[:, cs] * scale).astype(bf),
            "wk": np.ascontiguousarray(Wk[:, cs]).astype(bf),
            "wv": np.ascontiguousarray(Wv[:, cs]).astype(bf),
            "bq": np.ascontiguousarray((bq[cs] * scale).reshape(G, 1)),
            "bk": np.ascontiguousarray(bk[cs].reshape(G, 1)),
            "bv": np.ascontiguousarray(bv[cs]),
            "wo": np.ascontiguousarray(Wo[cs, :].reshape(NHL, 64, H)).astype(bf),
        })
    return in_maps


def kernel(x, Wq, bq, Wk, bk, Wv, bv, Wo, bo):
    from concourse.bass_utils import run_bass_kernel_spmd

    bo = np.asarray(bo, dtype=np.float32)
    nc = _build()
    in_maps = make_in_maps(x, Wq, bq, Wk, bk, Wv, bv, Wo)
    res = run_bass_kernel_spmd(nc, in_maps, core_ids=list(range(N_CORES)))

    out = np.empty((2, S, H), dtype=np.float32)
    for b in range(2):
        acc = res.results[4 * b]["out"].astype(np.float32)
        for g in range(1, 4):
            acc = acc + res.results[4 * b + g]["out"]
        out[b] = acc + bo
    return out



# revision 6
# speedup vs baseline: 1.0377x; 1.0377x over previous
"""Multi-head self-attention TRN2 Bass kernel (v2: PE-saturated schedule).

Problem: x[2, 2048, 1024], 16 heads x 64 dim, fp32.
Sharding: 8 cores = 2 batches x 4 head-groups (4 heads each).
Each core computes its batch's partial output (its 4 heads through
QKV -> attention -> output projection rows); host sums the 4 partials
per batch and adds bo.

The PE (tensor engine) is the bottleneck (~167us of streamed matmul
at bf16: scores 54.6 + attn@V 54.6 + QKV 41 + out_proj 13.6); the
ACT exp stream is ~142us. v2 therefore schedules for zero PE idle:

  - minimal warmup: only kT m=0 chunk 0 (no bias -- bk is dropped
    entirely: it shifts scores by a per-q constant, softmax-invariant),
    v chunks 0-3, qT m=0 chunk 0. Everything else streams into the
    attention sweep as dependency-placed inline fillers.
  - exp activation table pre-loaded during the initial DMA wait;
    dummy matmuls on a memset tile keep the PE p-state warm while
    the first x chunk lands.
  - scores->AV lag of 2 iterations so the in-order PE queue never
    head-stalls on the ACT stream.
  - normalize (evac + reciprocal + rank-1 PE broadcast) of pair p is
    deferred into pair p+1's early iterations; out_proj of q-chunk qc
    runs inside (qc+1, mt=0)'s sweep.
"""

import numpy as np

S = 2048          # sequence length per batch
H = 1024          # hidden
G = 256           # head-group width (4 heads x 64)
HD = 65           # V' columns per head (64 + ones)
NHL = 4           # heads per core
N_CORES = 8

_CACHE = {}


def _build():
    if "nc" in _CACHE:
        return _CACHE["nc"]

    import concourse.bass as bass
    import concourse.mybir as mybir
    import concourse.tile as tile
    from concourse import bacc
    from concourse.tile_rust import add_dep_helper

    f32 = mybir.dt.float32
    bf16 = mybir.dt.bfloat16
    EXP = mybir.ActivationFunctionType.Exp

    nc = bacc.Bacc("TRN2", target_bir_lowering=False, debug=False,
                   num_devices=N_CORES)

    xt_in = nc.dram_tensor("xt", [4, 128, 8, 512], bf16, kind="ExternalInput")
    wq_in = nc.dram_tensor("wq", [H, G], bf16, kind="ExternalInput")
    wk_in = nc.dram_tensor("wk", [H, G], bf16, kind="ExternalInput")
    wv_in = nc.dram_tensor("wv", [H, G], bf16, kind="ExternalInput")
    bq_in = nc.dram_tensor("bq", [G, 1], f32, kind="ExternalInput")
    bv_in = nc.dram_tensor("bv", [G], f32, kind="ExternalInput")
    wo_in = nc.dram_tensor("wo", [NHL, 64, H], bf16, kind="ExternalInput")
    out_d = nc.dram_tensor("out", [S, H], f32, kind="ExternalOutput")

    with tile.TileContext(nc) as tc:
        with tc.tile_pool(name="persist", bufs=1) as persist:
            qT = persist.tile([128, 2, S], bf16)     # [qd, m, s]
            kT = persist.tile([128, 2, S], bf16)
            vp = persist.tile([128, 16, NHL, HD], bf16)  # [s-part, st, h, col]
            bq_sb = persist.tile([128, 2, 1], f32)
            bv_bc = persist.tile([128, G], f32)
            wo_pr = persist.tile([128, 2, H], bf16)
            ones64 = persist.tile([1, 64], bf16)
            warm = persist.tile([128, 512], bf16)
            warm_e = persist.tile([1, 8], f32)
            wq_sb = persist.tile([128, 8, G], bf16)
            wk_sb = persist.tile([128, 8, G], bf16)
            wv_sb = persist.tile([128, 8, G], bf16)
            xTc = [persist.tile([128, 8, 512], bf16, name=f"xT_{jc}")
                   for jc in range(4)]

            with (
                tc.tile_pool(name="at_roll", bufs=2) as at_pool,
                tc.tile_pool(name="outP", bufs=4) as op_pool,
                tc.tile_pool(name="tmpo", bufs=1) as tmpo_pool,
                tc.tile_pool(name="sums", bufs=4) as sums_pool,
                tc.tile_pool(name="osb", bufs=2) as osb_pool,
                tc.tile_pool(name="ps_s", bufs=2, space="PSUM") as ps_s_pool,
                tc.tile_pool(name="ps_av", bufs=2, space="PSUM") as ps_av_pool,
                tc.tile_pool(name="ps_op", bufs=1, space="PSUM") as ps_op_pool,
            ):
                # ---------------- DMAs (priority chain) ----------------
                dma_wk = nc.sync.dma_start(
                    out=wk_sb, in_=wk_in.ap().rearrange("(t p) d -> p t d", p=128))
                x_dmas = [nc.sync.dma_start(out=xTc[jc], in_=xt_in.ap()[jc])
                          for jc in range(4)]
                dma_wv = nc.sync.dma_start(
                    out=wv_sb, in_=wv_in.ap().rearrange("(t p) d -> p t d", p=128))
                dma_wq = nc.sync.dma_start(
                    out=wq_sb, in_=wq_in.ap().rearrange("(t p) d -> p t d", p=128))
                nc.sync.dma_start(
                    out=bq_sb, in_=bq_in.ap().rearrange("(m p) o -> p m o", p=128))
                # broadcast bv along partitions (stride-0 partition AP)
                bv_ap = bass.AP(tensor=bv_in, offset=0, ap=[[0, 128], [1, G]])
                nc.gpsimd.dma_start(out=bv_bc, in_=bv_ap)
                # Wo as stacked head pairs: [two*64+p, pr, n]
                dma_wo = nc.sync.dma_start(
                    out=wo_pr,
                    in_=wo_in.ap().rearrange("(pr two) p n -> (two p) pr n", two=2))
                # priority: wk ∥ xc0 first, then wv, wq, then xc1-3, wo
                add_dep_helper(dma_wv.ins, x_dmas[0].ins, reason="dma order")
                add_dep_helper(dma_wq.ins, dma_wv.ins, reason="dma order")
                add_dep_helper(x_dmas[1].ins, dma_wq.ins, reason="dma order")
                add_dep_helper(x_dmas[2].ins, x_dmas[1].ins, reason="dma order")
                add_dep_helper(x_dmas[3].ins, x_dmas[2].ins, reason="dma order")
                add_dep_helper(dma_wo.ins, x_dmas[2].ins, reason="dma order")

                # memsets + p-state / act-table warmup during DMA wait
                nc.gpsimd.memset(warm, 0.125)
                nc.gpsimd.memset(vp[:, :, :, 64:65], 1.0)
                nc.gpsimd.memset(ones64, 1.0)
                # pre-load the exp activation table (~2.7us) off the
                # critical path
                nc.scalar.activation(out=warm_e, in_=warm[0:1, 0:8], func=EXP)

                def dummy(n=512):
                    ps_d = ps_op_pool.tile([128, 512], f32, tag="dummy",
                                           bufs=1)
                    nc.tensor.matmul(ps_d[:, 0:n], lhsT=warm[:, 0:128],
                                     rhs=warm[:, 0:n], start=True, stop=True)

                for _ in range(14):
                    dummy()

                # ---------------- QKV building blocks ----------------
                qk_ring = [0]

                def qk_full(w_sb, b_sb, dst, jc, m):
                    """dst[:, m, jc-chunk] = (x @ W)[:, m-half] (+ bias).

                    PSUM comes from the two 1-buf rings (dummy/oproj)
                    alternately so adjacent calls don't WAR-stall."""
                    sl = slice(jc * 512, (jc + 1) * 512)
                    tag = "dummy" if qk_ring[0] == 0 else "oproj"
                    qk_ring[0] ^= 1
                    ps = ps_op_pool.tile([128, 512], f32, tag=tag,
                                         name=f"psqk_{id(w_sb)}_{jc}_{m}",
                                         bufs=1)
                    for ht in range(8):
                        nc.tensor.matmul(
                            ps,
                            lhsT=w_sb[:, ht, m * 128:(m + 1) * 128],
                            rhs=xTc[jc][:, ht, :],
                            start=(ht == 0), stop=(ht == 7))
                    if b_sb is not None:
                        nc.vector.tensor_scalar_add(dst[:, m, sl], ps,
                                                    b_sb[:, m, :])
                    else:
                        nc.vector.tensor_copy(dst[:, m, sl], ps)

                def v_unit(st16):
                    tag = "dummy" if qk_ring[0] == 0 else "oproj"
                    qk_ring[0] ^= 1
                    ps_vt = ps_op_pool.tile([128, 512], f32, tag=tag,
                                            name=f"psv_{st16}", bufs=1)
                    for ht in range(8):
                        nc.tensor.matmul(
                            ps_vt[:, 0:G],
                            lhsT=xTc[st16 // 4][:, ht,
                                                (st16 % 4) * 128:
                                                (st16 % 4 + 1) * 128],
                            rhs=wv_sb[:, ht, :],
                            start=(ht == 0), stop=(ht == 7))
                    nc.vector.tensor_add(
                        vp[:, st16, :, 0:64],
                        ps_vt[:, 0:G].rearrange("p (h d) -> p h d", h=NHL),
                        bv_bc.rearrange("p (h d) -> p h d", h=NHL))

                # warmup: exactly what pair 0 iterations 0-3 need
                qk_full(wk_sb, None, kT, 0, 0)
                for i in range(4):
                    v_unit(i)
                qk_full(wq_sb, bq_sb, qT, 0, 0)

                # ---------------- normalize + out_proj ----------------
                def norm_evac(ps_av, hh, tag):
                    # evacuate PSUM right away to release the bank; MUST be
                    # emitted before the next pair's first AV matmul so the
                    # ring WAR dependency is seen
                    uout = tmpo_pool.tile([HD, 512], f32, tag="uout",
                                          name=f"uo_{tag}_{hh}", bufs=4)
                    nc.vector.tensor_copy(uout, ps_av)
                    return uout

                def norm_fin(outP, uout, hh, tag):
                    recip = sums_pool.tile([1, 512], f32, tag="recip",
                                           name=f"rc_{tag}_{hh}")
                    nc.vector.reciprocal_approx_fast(out=recip,
                                                     in_=uout[64:65, :])
                    recip_bf = sums_pool.tile([1, 512], bf16, tag="recipb",
                                              name=f"rcb_{tag}_{hh}")
                    nc.vector.tensor_copy(recip_bf, recip)
                    # broadcast along partitions: rank-1 outer product on
                    # the PE (ones[1,64].T @ recip[1,512] -> [64,512])
                    rbc = ps_op_pool.tile([64, 512], f32, tag="dummy",
                                          name=f"rb_{tag}_{hh}", bufs=1)
                    nc.tensor.matmul(rbc, lhsT=ones64, rhs=recip_bf,
                                     start=True, stop=True)
                    nc.vector.tensor_mul(
                        outP[hh * 64:hh * 64 + 64, :], uout[0:64, :], rbc)

                def oproj_unit(qc, outPs, qt, tail=False):
                    # out_proj for one q-tile (K=128 stacked pairs); at the
                    # kernel tail the freed score slots double-buffer it
                    osb = osb_pool.tile([128, H], f32, tag="osb",
                                        name=f"osb_{qc}_{qt}")
                    for ncx in range(2):
                        if tail:
                            ps_op = ps_s_pool.tile(
                                [128, 2, 512], f32, tag="s",
                                name=f"psot_{qc}_{qt}_{ncx}")[:, 0, :]
                        else:
                            ps_op = ps_op_pool.tile(
                                [128, 512], f32, tag="oproj",
                                name=f"pso_{qc}_{qt}_{ncx}", bufs=1)
                        for pr in range(2):
                            nc.tensor.matmul(
                                ps_op,
                                lhsT=outPs[pr][:, qt * 128:(qt + 1) * 128],
                                rhs=wo_pr[:, pr, ncx * 512:(ncx + 1) * 512],
                                start=(pr == 0), stop=(pr == 1))
                        nc.vector.tensor_copy(
                            osb[:, ncx * 512:(ncx + 1) * 512], ps_op)
                    nc.sync.dma_start(
                        out=out_d.ap()[qc * 512 + qt * 128:
                                       qc * 512 + (qt + 1) * 128, :],
                        in_=osb)

                # ---------------- attention sweep ----------------
                pending_norm = None   # (outP, ps_avs, tag) awaiting norm
                prev_oproj = None     # (qc, outPs) awaiting out_proj
                pair_idx = 0
                for qc in range(4):  # q-chunks of 512
                    qsl = slice(qc * 512, (qc + 1) * 512)
                    outPs = []
                    for mt in range(2):  # head pair (2mt, 2mt+1)
                        tag = f"{qc}_{mt}"
                        attnT = at_pool.tile([128, 2, 4, 512], bf16,
                                             tag="at", name=f"at_{tag}")
                        ps_avs = [ps_av_pool.tile([HD, 512], f32, tag="av",
                                                  name=f"av_{tag}_{hh}")
                                  for hh in range(2)]

                        def av_mm(kt, ps_avs=ps_avs, attnT=attnT, mt=mt):
                            for hh in range(2):
                                nc.tensor.matmul(
                                    ps_avs[hh],
                                    lhsT=vp[:, kt, 2 * mt + hh, :],
                                    rhs=attnT[:, hh, kt % 4, :],
                                    start=(kt == 0), stop=(kt == 15))

                        for kt in range(16):
                            # inline QKV fillers, placed just before need
                            if pair_idx == 0:
                                if kt % 4 == 0 and kt > 0:
                                    qk_full(wk_sb, None, kT, kt // 4, 0)
                                if 2 <= kt <= 13:
                                    v_unit(kt + 2)
                            elif pair_idx == 1 and kt % 4 == 0:
                                if kt == 0:
                                    qk_full(wq_sb, bq_sb, qT, 0, 1)
                                qk_full(wk_sb, None, kT, kt // 4, 1)
                            elif pair_idx >= 2 and kt == 0:
                                qk_full(wq_sb, bq_sb, qT, qc, mt)
                            # deferred normalize of the previous pair:
                            # evacuations at kt 0/1 (before av_mm(0) below),
                            # finish chains at kt 2/4
                            if pending_norm is not None:
                                pP, pavs, puo, ptag = pending_norm
                                if kt in (0, 1):
                                    puo.append(norm_evac(pavs[kt], kt, ptag))
                                elif kt == 2:
                                    norm_fin(pP, puo[0], 0, ptag)
                                elif kt == 4:
                                    norm_fin(pP, puo[1], 1, ptag)
                                    pending_norm = None
                            # deferred out_proj of the previous q-chunk
                            if prev_oproj is not None and kt in (6, 11):
                                pq, pouts = prev_oproj
                                qt0 = 0 if kt == 6 else 2
                                oproj_unit(pq, pouts, qt0)
                                oproj_unit(pq, pouts, qt0 + 1)
                                if kt == 11:
                                    prev_oproj = None
                            # scores (transposed): S^T[k,q] = kT.T @ qT
                            ps_s = ps_s_pool.tile([128, 2, 512], f32, tag="s")
                            for hh in range(2):
                                nc.tensor.matmul(
                                    ps_s[:, hh, :],
                                    lhsT=kT[hh * 64:hh * 64 + 64, mt,
                                            kt * 128:(kt + 1) * 128],
                                    rhs=qT[hh * 64:hh * 64 + 64, mt, qsl],
                                    start=True, stop=True)
                            nc.scalar.activation(
                                out=attnT[:, :, kt % 4, :], in_=ps_s, func=EXP)
                            if kt >= 2:
                                av_mm(kt - 2)
                        av_mm(14)
                        av_mm(15)

                        outP = op_pool.tile([128, 512], bf16, tag="outP",
                                            name=f"outP_{tag}")
                        pending_norm = (outP, ps_avs, [], tag)
                        outPs.append(outP)
                        pair_idx += 1
                    prev_oproj = (qc, outPs)

                # tail: last pair's normalize, then final out_proj
                pP, pavs, puo, ptag = pending_norm
                for hh in range(2):
                    puo.append(norm_evac(pavs[hh], hh, ptag))
                for hh in range(2):
                    norm_fin(pP, puo[hh], hh, ptag)
                pq, pouts = prev_oproj
                for qt in range(4):
                    oproj_unit(pq, pouts, qt, tail=True)

    nc.compile()
    _CACHE["nc"] = nc
    return nc


def make_in_maps(x, Wq, bq, Wk, bk, Wv, bv, Wo):
    import ml_dtypes
    bf = ml_dtypes.bfloat16

    x = np.asarray(x, dtype=np.float32)
    Wq = np.asarray(Wq, dtype=np.float32)
    bq = np.asarray(bq, dtype=np.float32)
    Wk = np.asarray(Wk, dtype=np.float32)
    Wv = np.asarray(Wv, dtype=np.float32)
    bv = np.asarray(bv, dtype=np.float32)
    Wo = np.asarray(Wo, dtype=np.float32)

    scale = np.float32(1.0 / 8.0)  # 1/sqrt(64)

    in_maps = []
    for core in range(N_CORES):
        b = core // 4
        g = core % 4
        cs = slice(g * G, (g + 1) * G)
        in_maps.append({
            "xt": np.ascontiguousarray(
                x[b].reshape(4, 512, 8, 128).transpose(0, 3, 2, 1)).astype(bf),
            "wq": np.ascontiguousarray(Wq[:, cs] * scale).astype(bf),
            "wk": np.ascontiguousarray(Wk[:, cs]).astype(bf),
            "wv": np.ascontiguousarray(Wv[:, cs]).astype(bf),
            "bq": np.ascontiguousarray((bq[cs] * scale).reshape(G, 1)),
            "bv": np.ascontiguousarray(bv[cs]),
            "wo": np.ascontiguousarray(Wo[cs, :].reshape(NHL, 64, H)).astype(bf),
        })
    return in_maps


def kernel(x, Wq, bq, Wk, bk, Wv, bv, Wo, bo):
    from concourse.bass_utils import run_bass_kernel_spmd

    bo = np.asarray(bo, dtype=np.float32)
    nc = _build()
    in_maps = make_in_maps(x, Wq, bq, Wk, bk, Wv, bv, Wo)
    res = run_bass_kernel_spmd(nc, in_maps, core_ids=list(range(N_CORES)))

    out = np.empty((2, S, H), dtype=np.float32)
    for b in range(2):
        acc = res.results[4 * b]["out"].astype(np.float32)
        for g in range(1, 4):
            acc = acc + res.results[4 * b + g]["out"]
        out[b] = acc + bo
    return out


# revision 10
# speedup vs baseline: 1.0459x; 1.0078x over previous
"""Multi-head self-attention TRN2 Bass kernel (v2: PE-saturated schedule).

Problem: x[2, 2048, 1024], 16 heads x 64 dim, fp32.
Sharding: 8 cores = 2 batches x 4 head-groups (4 heads each).
Each core computes its batch's partial output (its 4 heads through
QKV -> attention -> output projection rows); host sums the 4 partials
per batch and adds bo.

The PE (tensor engine) is the bottleneck (~167us of streamed matmul
at bf16: scores 54.6 + attn@V 54.6 + QKV 41 + out_proj 13.6); the
ACT exp stream is ~142us. v2 therefore schedules for zero PE idle:

  - minimal warmup: only kT m=0 chunk 0 (no bias -- bk is dropped
    entirely: it shifts scores by a per-q constant, softmax-invariant),
    v chunks 0-3, qT m=0 chunk 0. Everything else streams into the
    attention sweep as dependency-placed inline fillers.
  - exp activation table pre-loaded during the initial DMA wait;
    dummy matmuls on a memset tile keep the PE p-state warm while
    the first x chunk lands.
  - scores->AV lag of 2 iterations so the in-order PE queue never
    head-stalls on the ACT stream.
  - normalize (evac + reciprocal + rank-1 PE broadcast) of pair p is
    deferred into pair p+1's early iterations; out_proj of q-chunk qc
    runs inside (qc+1, mt=0)'s sweep.
"""

import numpy as np

S = 2048          # sequence length per batch
H = 1024          # hidden
G = 256           # head-group width (4 heads x 64)
HD = 65           # V' columns per head (64 + ones)
NHL = 4           # heads per core
N_CORES = 8

_CACHE = {}


def _build():
    if "nc" in _CACHE:
        return _CACHE["nc"]

    import concourse.bass as bass
    import concourse.mybir as mybir
    import concourse.tile as tile
    from concourse import bacc
    from concourse.tile_rust import add_dep_helper

    f32 = mybir.dt.float32
    bf16 = mybir.dt.bfloat16
    EXP = mybir.ActivationFunctionType.Exp

    nc = bacc.Bacc("TRN2", target_bir_lowering=False, debug=False,
                   num_devices=N_CORES)

    xt_in = nc.dram_tensor("xt", [4, 128, 8, 512], bf16, kind="ExternalInput")
    wq_in = nc.dram_tensor("wq", [H, G], bf16, kind="ExternalInput")
    wk_in = nc.dram_tensor("wk", [H, G], bf16, kind="ExternalInput")
    wv_in = nc.dram_tensor("wv", [H, G], bf16, kind="ExternalInput")
    bq_in = nc.dram_tensor("bq", [G, 1], f32, kind="ExternalInput")
    bv_in = nc.dram_tensor("bv", [G], f32, kind="ExternalInput")
    wo_in = nc.dram_tensor("wo", [NHL, 64, H], bf16, kind="ExternalInput")
    out_d = nc.dram_tensor("out", [S, H], f32, kind="ExternalOutput")

    with tile.TileContext(nc) as tc:
        with tc.tile_pool(name="persist", bufs=1) as persist:
            qT = persist.tile([128, 2, S], bf16)     # [qd, m, s]
            kT = persist.tile([128, 2, S], bf16)
            vp = persist.tile([128, 16, NHL, HD], bf16)  # [s-part, st, h, col]
            bq_sb = persist.tile([128, 2, 1], f32)
            bv_f = persist.tile([1, G], f32)
            bv_bf = persist.tile([1, G], bf16)
            bv_bc = persist.tile([128, G], f32)
            wo_pr = persist.tile([128, 2, H], bf16)
            ones64 = persist.tile([1, 64], bf16)
            ones128 = persist.tile([1, 128], bf16)
            warm = persist.tile([128, 512], bf16)
            warm_e = persist.tile([1, 8], f32)
            wq_sb = persist.tile([128, 8, G], bf16)
            wk_sb = persist.tile([128, 8, G], bf16)
            wv_sb = persist.tile([128, 8, G], bf16)
            xTc = [persist.tile([128, 8, 512], bf16, name=f"xT_{jc}")
                   for jc in range(4)]

            with (
                tc.tile_pool(name="at_roll", bufs=2) as at_pool,
                tc.tile_pool(name="outP", bufs=4) as op_pool,
                tc.tile_pool(name="tmpo", bufs=1) as tmpo_pool,
                tc.tile_pool(name="sums", bufs=4) as sums_pool,
                tc.tile_pool(name="osb", bufs=2) as osb_pool,
                tc.tile_pool(name="ps_s", bufs=2, space="PSUM") as ps_s_pool,
                tc.tile_pool(name="ps_av", bufs=2, space="PSUM") as ps_av_pool,
                tc.tile_pool(name="ps_op", bufs=1, space="PSUM") as ps_op_pool,
            ):
                # memsets first so the gpsimd queue isn't blocked
                nc.gpsimd.memset(warm, 0.125)
                nc.gpsimd.memset(vp[:, :, :, 64:65], 1.0)
                nc.gpsimd.memset(ones64, 1.0)
                nc.gpsimd.memset(ones128, 1.0)

                # ---------------- DMAs (two priority chains) ----------------
                # chain A (small): bv -> wk -> wv -> wq -> bq
                dma_bv = nc.sync.dma_start(
                    out=bv_f, in_=bv_in.ap().rearrange("(o g) -> o g", o=1))
                dma_wk = nc.sync.dma_start(
                    out=wk_sb, in_=wk_in.ap().rearrange("(t p) d -> p t d", p=128))
                dma_wv = nc.sync.dma_start(
                    out=wv_sb, in_=wv_in.ap().rearrange("(t p) d -> p t d", p=128))
                dma_wq = nc.sync.dma_start(
                    out=wq_sb, in_=wq_in.ap().rearrange("(t p) d -> p t d", p=128))
                dma_bq = nc.sync.dma_start(
                    out=bq_sb, in_=bq_in.ap().rearrange("(m p) o -> p m o", p=128))
                # chain B (big): xc0 -> xc1 -> xc2 -> xc3 -> wo
                x_dmas = [nc.sync.dma_start(out=xTc[jc], in_=xt_in.ap()[jc])
                          for jc in range(4)]
                # Wo as stacked head pairs: [two*64+p, pr, n]
                dma_wo = nc.sync.dma_start(
                    out=wo_pr,
                    in_=wo_in.ap().rearrange("(pr two) p n -> (two p) pr n", two=2))
                for a, b in [(dma_wk, dma_bv), (dma_wv, dma_wk),
                             (dma_wq, dma_wv), (dma_bq, dma_wq),
                             (x_dmas[1], x_dmas[0]), (x_dmas[2], x_dmas[1]),
                             (x_dmas[3], x_dmas[2]), (dma_wo, x_dmas[3])]:
                    add_dep_helper(a.ins, b.ins, reason="dma order")

                # pre-load the exp activation table (~2.7us) off the
                # critical path
                nc.scalar.activation(out=warm_e, in_=warm[0:1, 0:8], func=EXP)

                def dummy(n=512):
                    ps_d = ps_op_pool.tile([128, 512], f32, tag="dummy",
                                           bufs=1)
                    nc.tensor.matmul(ps_d[:, 0:n], lhsT=warm[:, 0:128],
                                     rhs=warm[:, 0:n], start=True, stop=True)

                for _ in range(4):
                    dummy()
                # bv broadcast along partitions via rank-1 PE outer product
                nc.vector.tensor_copy(bv_bf, bv_f)
                ps_bv = ps_op_pool.tile([128, G], f32, tag="oproj",
                                        name="ps_bv", bufs=1)
                nc.tensor.matmul(ps_bv, lhsT=ones128, rhs=bv_bf,
                                 start=True, stop=True)
                nc.vector.tensor_copy(bv_bc, ps_bv)
                for _ in range(4):
                    dummy()

                # ---------------- QKV building blocks ----------------
                qk_ring = [0]

                def qk_full(w_sb, b_sb, dst, jc, m):
                    """dst[:, m, jc-chunk] = (x @ W)[:, m-half] (+ bias).

                    PSUM comes from the two 1-buf rings (dummy/oproj)
                    alternately so adjacent calls don't WAR-stall."""
                    sl = slice(jc * 512, (jc + 1) * 512)
                    tag = "dummy" if qk_ring[0] == 0 else "oproj"
                    qk_ring[0] ^= 1
                    ps = ps_op_pool.tile([128, 512], f32, tag=tag,
                                         name=f"psqk_{id(w_sb)}_{jc}_{m}",
                                         bufs=1)
                    for ht in range(8):
                        nc.tensor.matmul(
                            ps,
                            lhsT=w_sb[:, ht, m * 128:(m + 1) * 128],
                            rhs=xTc[jc][:, ht, :],
                            start=(ht == 0), stop=(ht == 7))
                    if b_sb is not None:
                        nc.vector.tensor_scalar_add(dst[:, m, sl], ps,
                                                    b_sb[:, m, :])
                    else:
                        nc.vector.tensor_copy(dst[:, m, sl], ps)

                def v_unit(st16):
                    tag = "dummy" if qk_ring[0] == 0 else "oproj"
                    qk_ring[0] ^= 1
                    ps_vt = ps_op_pool.tile([128, 512], f32, tag=tag,
                                            name=f"psv_{st16}", bufs=1)
                    for ht in range(8):
                        nc.tensor.matmul(
                            ps_vt[:, 0:G],
                            lhsT=xTc[st16 // 4][:, ht,
                                                (st16 % 4) * 128:
                                                (st16 % 4 + 1) * 128],
                            rhs=wv_sb[:, ht, :],
                            start=(ht == 0), stop=(ht == 7))
                    nc.vector.tensor_add(
                        vp[:, st16, :, 0:64],
                        ps_vt[:, 0:G].rearrange("p (h d) -> p h d", h=NHL),
                        bv_bc.rearrange("p (h d) -> p h d", h=NHL))

                # warmup: exactly what pair 0 iterations 0-3 need
                qk_full(wk_sb, None, kT, 0, 0)
                for i in range(4):
                    v_unit(i)
                qk_full(wq_sb, bq_sb, qT, 0, 0)

                # ---------------- normalize + out_proj ----------------
                def norm_evac(ps_av, hh, tag):
                    # evacuate PSUM right away to release the bank; MUST be
                    # emitted before the next pair's first AV matmul so the
                    # ring WAR dependency is seen
                    uout = tmpo_pool.tile([HD, 512], f32, tag="uout",
                                          name=f"uo_{tag}_{hh}", bufs=4)
                    nc.vector.tensor_copy(uout, ps_av)
                    return uout

                def norm_fin(outP, uout, hh, tag):
                    sums = sums_pool.tile([1, 512], f32, tag="sums",
                                          name=f"sm_{tag}_{hh}")
                    nc.vector.tensor_copy(sums, uout[64:65, :])
                    recip = sums_pool.tile([1, 512], f32, tag="recip",
                                           name=f"rc_{tag}_{hh}")
                    nc.vector.reciprocal_approx_fast(out=recip, in_=sums)
                    recip_bf = sums_pool.tile([1, 512], bf16, tag="recipb",
                                              name=f"rcb_{tag}_{hh}")
                    nc.vector.tensor_copy(recip_bf, recip)
                    # broadcast along partitions: rank-1 outer product on
                    # the PE (ones[1,64].T @ recip[1,512] -> [64,512])
                    rbc = ps_op_pool.tile([64, 512], f32, tag="dummy",
                                          name=f"rb_{tag}_{hh}", bufs=1)
                    nc.tensor.matmul(rbc, lhsT=ones64, rhs=recip_bf,
                                     start=True, stop=True)
                    nc.vector.tensor_mul(
                        outP[hh * 64:hh * 64 + 64, :], uout[0:64, :], rbc)

                def oproj_unit(qc, outPs, qt, tail=False):
                    # out_proj for one q-tile (K=128 stacked pairs); at the
                    # kernel tail the freed score slots double-buffer it
                    osb = osb_pool.tile([128, H], f32, tag="osb",
                                        name=f"osb_{qc}_{qt}")
                    for ncx in range(2):
                        if tail:
                            ps_op = ps_s_pool.tile(
                                [128, 2, 512], f32, tag="s",
                                name=f"psot_{qc}_{qt}_{ncx}")[:, 0, :]
                        else:
                            ps_op = ps_op_pool.tile(
                                [128, 512], f32, tag="oproj",
                                name=f"pso_{qc}_{qt}_{ncx}", bufs=1)
                        for pr in range(2):
                            nc.tensor.matmul(
                                ps_op,
                                lhsT=outPs[pr][:, qt * 128:(qt + 1) * 128],
                                rhs=wo_pr[:, pr, ncx * 512:(ncx + 1) * 512],
                                start=(pr == 0), stop=(pr == 1))
                        nc.vector.tensor_copy(
                            osb[:, ncx * 512:(ncx + 1) * 512], ps_op)
                    nc.sync.dma_start(
                        out=out_d.ap()[qc * 512 + qt * 128:
                                       qc * 512 + (qt + 1) * 128, :],
                        in_=osb)

                # ---------------- attention sweep ----------------
                pending_norm = None   # (outP, ps_avs, tag) awaiting norm
                prev_oproj = None     # (qc, outPs) awaiting out_proj
                pair_idx = 0
                for qc in range(4):  # q-chunks of 512
                    qsl = slice(qc * 512, (qc + 1) * 512)
                    outPs = []
                    for mt in range(2):  # head pair (2mt, 2mt+1)
                        tag = f"{qc}_{mt}"
                        attnT = at_pool.tile([128, 2, 4, 512], bf16,
                                             tag="at", name=f"at_{tag}")
                        ps_avs = [ps_av_pool.tile([HD, 512], f32, tag="av",
                                                  name=f"av_{tag}_{hh}")
                                  for hh in range(2)]

                        def av_mm(kt, ps_avs=ps_avs, attnT=attnT, mt=mt):
                            for hh in range(2):
                                nc.tensor.matmul(
                                    ps_avs[hh],
                                    lhsT=vp[:, kt, 2 * mt + hh, :],
                                    rhs=attnT[:, hh, kt % 4, :],
                                    start=(kt == 0), stop=(kt == 15))

                        for kt in range(16):
                            # inline QKV fillers, placed just before need
                            if pair_idx == 0:
                                if kt % 4 == 0 and kt > 0:
                                    qk_full(wk_sb, None, kT, kt // 4, 0)
                                if 2 <= kt <= 13:
                                    v_unit(kt + 2)
                            elif pair_idx == 1 and kt % 4 == 0:
                                if kt == 0:
                                    qk_full(wq_sb, bq_sb, qT, 0, 1)
                                qk_full(wk_sb, None, kT, kt // 4, 1)
                            elif pair_idx >= 2 and kt == 0:
                                qk_full(wq_sb, bq_sb, qT, qc, mt)
                            # deferred normalize of the previous pair:
                            # evacuations at kt 0/1 (before av_mm(0) below),
                            # finish chains at kt 2/4
                            if pending_norm is not None:
                                pP, pavs, puo, ptag = pending_norm
                                if kt in (0, 1):
                                    puo.append(norm_evac(pavs[kt], kt, ptag))
                                elif kt == 2:
                                    norm_fin(pP, puo[0], 0, ptag)
                                elif kt == 4:
                                    norm_fin(pP, puo[1], 1, ptag)
                                    pending_norm = None
                            # deferred out_proj of the previous q-chunk
                            if prev_oproj is not None and kt in (6, 11):
                                pq, pouts = prev_oproj
                                qt0 = 0 if kt == 6 else 2
                                oproj_unit(pq, pouts, qt0)
                                oproj_unit(pq, pouts, qt0 + 1)
                                if kt == 11:
                                    prev_oproj = None
                            # scores (transposed): S^T[k,q] = kT.T @ qT
                            ps_s = ps_s_pool.tile([128, 2, 512], f32, tag="s")
                            for hh in range(2):
                                nc.tensor.matmul(
                                    ps_s[:, hh, :],
                                    lhsT=kT[hh * 64:hh * 64 + 64, mt,
                                            kt * 128:(kt + 1) * 128],
                                    rhs=qT[hh * 64:hh * 64 + 64, mt, qsl],
                                    start=True, stop=True)
                            nc.scalar.activation(
                                out=attnT[:, :, kt % 4, :], in_=ps_s, func=EXP)
                            if kt >= 2:
                                av_mm(kt - 2)
                        av_mm(14)
                        av_mm(15)

                        outP = op_pool.tile([128, 512], bf16, tag="outP",
                                            name=f"outP_{tag}")
                        pending_norm = (outP, ps_avs, [], tag)
                        outPs.append(outP)
                        pair_idx += 1
                    prev_oproj = (qc, outPs)

                # tail: last pair's normalize, then final out_proj
                pP, pavs, puo, ptag = pending_norm
                for hh in range(2):
                    puo.append(norm_evac(pavs[hh], hh, ptag))
                for hh in range(2):
                    norm_fin(pP, puo[hh], hh, ptag)
                pq, pouts = prev_oproj
                for qt in range(4):
                    oproj_unit(pq, pouts, qt, tail=True)

    nc.compile()
    _CACHE["nc"] = nc
    return nc


def make_in_maps(x, Wq, bq, Wk, bk, Wv, bv, Wo):
    import ml_dtypes
    bf = ml_dtypes.bfloat16

    x = np.asarray(x, dtype=np.float32)
    Wq = np.asarray(Wq, dtype=np.float32)
    bq = np.asarray(bq, dtype=np.float32)
    Wk = np.asarray(Wk, dtype=np.float32)
    Wv = np.asarray(Wv, dtype=np.float32)
    bv = np.asarray(bv, dtype=np.float32)
    Wo = np.asarray(Wo, dtype=np.float32)

    scale = np.float32(1.0 / 8.0)  # 1/sqrt(64)

    in_maps = []
    for core in range(N_CORES):
        b = core // 4
        g = core % 4
        cs = slice(g * G, (g + 1) * G)
        in_maps.append({
            "xt": np.ascontiguousarray(
                x[b].reshape(4, 512, 8, 128).transpose(0, 3, 2, 1)).astype(bf),
            "wq": np.ascontiguousarray(Wq[:, cs] * scale).astype(bf),
            "wk": np.ascontiguousarray(Wk[:, cs]).astype(bf),
            "wv": np.ascontiguousarray(Wv[:, cs]).astype(bf),
            "bq": np.ascontiguousarray((bq[cs] * scale).reshape(G, 1)),
            "bv": np.ascontiguousarray(bv[cs]),
            "wo": np.ascontiguousarray(Wo[cs, :].reshape(NHL, 64, H)).astype(bf),
        })
    return in_maps


def kernel(x, Wq, bq, Wk, bk, Wv, bv, Wo, bo):
    from concourse.bass_utils import run_bass_kernel_spmd

    bo = np.asarray(bo, dtype=np.float32)
    nc = _build()
    in_maps = make_in_maps(x, Wq, bq, Wk, bk, Wv, bv, Wo)
    res = run_bass_kernel_spmd(nc, in_maps, core_ids=list(range(N_CORES)))

    out = np.empty((2, S, H), dtype=np.float32)
    for b in range(2):
        acc = res.results[4 * b]["out"].astype(np.float32)
        for g in range(1, 4):
            acc = acc + res.results[4 * b + g]["out"]
        out[b] = acc + bo
    return out


# revision 17
# speedup vs baseline: 1.0582x; 1.0118x over previous
"""Multi-head self-attention TRN2 Bass kernel (v2: PE-saturated schedule).

Problem: x[2, 2048, 1024], 16 heads x 64 dim, fp32.
Sharding: 8 cores = 2 batches x 4 head-groups (4 heads each).
Each core computes its batch's partial output (its 4 heads through
QKV -> attention -> output projection rows); host sums the 4 partials
per batch and adds bo.

The PE (tensor engine) is the bottleneck (~167us of streamed matmul
at bf16: scores 54.6 + attn@V 54.6 + QKV 41 + out_proj 13.6); the
ACT exp stream is ~142us. v2 therefore schedules for zero PE idle:

  - minimal warmup: only kT m=0 chunk 0 (no bias -- bk is dropped
    entirely: it shifts scores by a per-q constant, softmax-invariant),
    v chunks 0-3, qT m=0 chunk 0. Everything else streams into the
    attention sweep as dependency-placed inline fillers.
  - exp activation table pre-loaded during the initial DMA wait;
    dummy matmuls on a memset tile keep the PE p-state warm while
    the first x chunk lands.
  - scores->AV lag of 2 iterations so the in-order PE queue never
    head-stalls on the ACT stream.
  - normalize (evac + reciprocal + rank-1 PE broadcast) of pair p is
    deferred into pair p+1's early iterations; out_proj of q-chunk qc
    runs inside (qc+1, mt=0)'s sweep.
"""

import numpy as np

S = 2048          # sequence length per batch
H = 1024          # hidden
G = 256           # head-group width (4 heads x 64)
HD = 65           # V' columns per head (64 + ones)
NHL = 4           # heads per core
N_CORES = 8

_CACHE = {}


def _build():
    if "nc" in _CACHE:
        return _CACHE["nc"]

    import concourse.bass as bass
    import concourse.mybir as mybir
    import concourse.tile as tile
    from concourse import bacc
    from concourse.tile_rust import add_dep_helper

    f32 = mybir.dt.float32
    bf16 = mybir.dt.bfloat16
    EXP = mybir.ActivationFunctionType.Exp

    nc = bacc.Bacc("TRN2", target_bir_lowering=False, debug=False,
                   num_devices=N_CORES)

    xt_in = nc.dram_tensor("xt", [4, 128, 8, 512], bf16, kind="ExternalInput")
    wq_in = nc.dram_tensor("wq", [H, G], bf16, kind="ExternalInput")
    wk_in = nc.dram_tensor("wk", [H, G], bf16, kind="ExternalInput")
    wv_in = nc.dram_tensor("wv", [H, G], bf16, kind="ExternalInput")
    bq_in = nc.dram_tensor("bq", [G, 1], f32, kind="ExternalInput")
    bv_in = nc.dram_tensor("bv", [G], f32, kind="ExternalInput")
    wo_in = nc.dram_tensor("wo", [NHL, 64, H], bf16, kind="ExternalInput")
    out_d = nc.dram_tensor("out", [S, H], f32, kind="ExternalOutput")

    with tile.TileContext(nc) as tc:
        with tc.tile_pool(name="persist", bufs=1) as persist:
            qT = persist.tile([128, 2, S], bf16)     # [qd, m, s]
            kT = persist.tile([128, 2, S], bf16)
            vp = persist.tile([128, 16, NHL, HD], bf16)  # [s-part, st, h, col]
            bq_sb = persist.tile([128, 2, 1], f32)
            bv_f = persist.tile([1, G], f32)
            bv_bf = persist.tile([1, G], bf16)
            bv_bc = persist.tile([128, G], f32)
            wo_pr = persist.tile([128, 2, H], bf16)
            ones64 = persist.tile([1, 64], bf16)
            ones128 = persist.tile([1, 128], bf16)
            warm = persist.tile([128, 512], bf16)
            warm_e = persist.tile([1, 8], f32)
            wq_sb = persist.tile([128, 8, G], bf16)
            wk_sb = persist.tile([128, 8, G], bf16)
            wv_sb = persist.tile([128, 8, G], bf16)
            xTc = [persist.tile([128, 8, 512], bf16, name=f"xT_{jc}")
                   for jc in range(4)]

            with (
                tc.tile_pool(name="at_roll", bufs=2) as at_pool,
                tc.tile_pool(name="outP", bufs=4) as op_pool,
                tc.tile_pool(name="tmpo", bufs=1) as tmpo_pool,
                tc.tile_pool(name="sums", bufs=4) as sums_pool,
                tc.tile_pool(name="osb", bufs=2) as osb_pool,
                tc.tile_pool(name="ps_s", bufs=2, space="PSUM") as ps_s_pool,
                tc.tile_pool(name="ps_av", bufs=2, space="PSUM") as ps_av_pool,
                tc.tile_pool(name="ps_op", bufs=1, space="PSUM") as ps_op_pool,
            ):
                # memsets first so the gpsimd queue isn't blocked
                nc.gpsimd.memset(warm, 0.125)
                nc.gpsimd.memset(vp[:, :, :, 64:65], 1.0)
                nc.gpsimd.memset(ones64, 1.0)
                nc.gpsimd.memset(ones128, 1.0)

                # ---------------- DMAs (two priority chains) ----------------
                # chain A (small): bv -> wk -> wv -> wq -> bq
                dma_bv = nc.sync.dma_start(
                    out=bv_f, in_=bv_in.ap().rearrange("(o g) -> o g", o=1))
                dma_wk = nc.sync.dma_start(
                    out=wk_sb, in_=wk_in.ap().rearrange("(t p) d -> p t d", p=128))
                dma_wv = nc.sync.dma_start(
                    out=wv_sb, in_=wv_in.ap().rearrange("(t p) d -> p t d", p=128))
                dma_wq = nc.sync.dma_start(
                    out=wq_sb, in_=wq_in.ap().rearrange("(t p) d -> p t d", p=128))
                dma_bq = nc.sync.dma_start(
                    out=bq_sb, in_=bq_in.ap().rearrange("(m p) o -> p m o", p=128))
                # chain B (big): xc0 (split in halves so the warmup matmuls
                # can start after 512KB) -> xc1 -> xc2 -> xc3 -> wo
                x0a = nc.sync.dma_start(out=xTc[0][:, 0:4, :],
                                        in_=xt_in.ap()[0][:, 0:4, :])
                x0b = nc.sync.dma_start(out=xTc[0][:, 4:8, :],
                                        in_=xt_in.ap()[0][:, 4:8, :])
                x_dmas = [x0b] + [nc.sync.dma_start(out=xTc[jc],
                                                    in_=xt_in.ap()[jc])
                          for jc in range(1, 4)]
                add_dep_helper(x0b.ins, x0a.ins, reason="dma order")
                # Wo as stacked head pairs: [two*64+p, pr, n]
                dma_wo = nc.sync.dma_start(
                    out=wo_pr,
                    in_=wo_in.ap().rearrange("(pr two) p n -> (two p) pr n", two=2))
                for a, b in [(dma_wk, dma_bv), (dma_wv, dma_wk),
                             (dma_wq, dma_wv), (dma_bq, dma_wq),
                             (x_dmas[1], x_dmas[0]), (x_dmas[2], x_dmas[1]),
                             (x_dmas[3], x_dmas[2]), (dma_wo, x_dmas[3])]:
                    add_dep_helper(a.ins, b.ins, reason="dma order")

                # pre-load the exp activation table (~2.7us) off the
                # critical path
                nc.scalar.activation(out=warm_e, in_=warm[0:1, 0:8], func=EXP)

                def dummy(n=512):
                    ps_d = ps_op_pool.tile([128, 512], f32, tag="dummy",
                                           bufs=1)
                    nc.tensor.matmul(ps_d[:, 0:n], lhsT=warm[:, 0:128],
                                     rhs=warm[:, 0:n], start=True, stop=True)

                for _ in range(4):
                    dummy()
                # bv broadcast along partitions via rank-1 PE outer product
                nc.vector.tensor_copy(bv_bf, bv_f)
                ps_bv = ps_op_pool.tile([128, G], f32, tag="oproj",
                                        name="ps_bv", bufs=1)
                nc.tensor.matmul(ps_bv, lhsT=ones128, rhs=bv_bf,
                                 start=True, stop=True)
                nc.vector.tensor_copy(bv_bc, ps_bv)
                for _ in range(4):
                    dummy()

                # ---------------- QKV building blocks ----------------
                qk_ring = [0]

                def qk_full(w_sb, b_sb, dst, jc, m):
                    """dst[:, m, jc-chunk] = (x @ W)[:, m-half] (+ bias).

                    PSUM comes from the two 1-buf rings (dummy/oproj)
                    alternately so adjacent calls don't WAR-stall."""
                    sl = slice(jc * 512, (jc + 1) * 512)
                    tag = "dummy" if qk_ring[0] == 0 else "oproj"
                    qk_ring[0] ^= 1
                    ps = ps_op_pool.tile([128, 512], f32, tag=tag,
                                         name=f"psqk_{id(w_sb)}_{jc}_{m}",
                                         bufs=1)
                    for ht in range(8):
                        nc.tensor.matmul(
                            ps,
                            lhsT=w_sb[:, ht, m * 128:(m + 1) * 128],
                            rhs=xTc[jc][:, ht, :],
                            start=(ht == 0), stop=(ht == 7))
                    if b_sb is not None:
                        nc.vector.tensor_scalar_add(dst[:, m, sl], ps,
                                                    b_sb[:, m, :])
                    else:
                        nc.vector.tensor_copy(dst[:, m, sl], ps)

                def v_unit(st16):
                    tag = "dummy" if qk_ring[0] == 0 else "oproj"
                    qk_ring[0] ^= 1
                    ps_vt = ps_op_pool.tile([128, 512], f32, tag=tag,
                                            name=f"psv_{st16}", bufs=1)
                    for ht in range(8):
                        nc.tensor.matmul(
                            ps_vt[:, 0:G],
                            lhsT=xTc[st16 // 4][:, ht,
                                                (st16 % 4) * 128:
                                                (st16 % 4 + 1) * 128],
                            rhs=wv_sb[:, ht, :],
                            start=(ht == 0), stop=(ht == 7))
                    nc.vector.tensor_add(
                        vp[:, st16, :, 0:64],
                        ps_vt[:, 0:G].rearrange("p (h d) -> p h d", h=NHL),
                        bv_bc.rearrange("p (h d) -> p h d", h=NHL))

                # warmup: exactly what pair 0 iterations 0-3 need
                qk_full(wk_sb, None, kT, 0, 0)
                for i in range(4):
                    v_unit(i)
                qk_full(wq_sb, bq_sb, qT, 0, 0)

                # ---------------- normalize + out_proj ----------------
                def norm_evac(ps_av, hh, tag):
                    # evacuate PSUM right away to release the bank; MUST be
                    # emitted before the next pair's first AV matmul so the
                    # ring WAR dependency is seen
                    uout = tmpo_pool.tile([HD, 512], f32, tag="uout",
                                          name=f"uo_{tag}_{hh}", bufs=4)
                    nc.vector.tensor_copy(uout, ps_av)
                    return uout

                def norm_recip(uout, hh, tag):
                    # DVE chain: sums copy -> fast reciprocal -> bf16 cast
                    sums = sums_pool.tile([1, 512], f32, tag="sums",
                                          name=f"sm_{tag}_{hh}")
                    nc.vector.tensor_copy(sums, uout[64:65, :])
                    recip = sums_pool.tile([1, 512], f32, tag="recip",
                                           name=f"rc_{tag}_{hh}")
                    nc.vector.reciprocal_approx_fast(out=recip, in_=sums)
                    recip_bf = sums_pool.tile([1, 512], bf16, tag="recipb",
                                              name=f"rcb_{tag}_{hh}")
                    nc.vector.tensor_copy(recip_bf, recip)
                    return recip_bf

                def norm_fin(outP, uout, recip_bf, hh, tag):
                    # broadcast along partitions: rank-1 outer product on
                    # the PE (ones[1,64].T @ recip[1,512] -> [64,512])
                    rbc = ps_op_pool.tile([64, 512], f32, tag="dummy",
                                          name=f"rb_{tag}_{hh}", bufs=1)
                    nc.tensor.matmul(rbc, lhsT=ones64, rhs=recip_bf,
                                     start=True, stop=True)
                    nc.vector.tensor_mul(
                        outP[hh * 64:hh * 64 + 64, :], uout[0:64, :], rbc)

                def oproj_unit(qc, outPs, qt, tail=False):
                    # out_proj for one q-tile (K=128 stacked pairs); the two
                    # ncx halves alternate the 1-buf rings so ncx1's matmuls
                    # don't WAR-stall on ncx0's evacuation; at the kernel
                    # tail the freed score slots double-buffer it
                    osb = osb_pool.tile([128, H], f32, tag="osb",
                                        name=f"osb_{qc}_{qt}")
                    for ncx in range(2):
                        if tail:
                            ps_op = ps_s_pool.tile(
                                [128, 2, 512], f32, tag="s",
                                name=f"psot_{qc}_{qt}_{ncx}")[:, 0, :]
                        else:
                            ps_op = ps_op_pool.tile(
                                [128, 512], f32,
                                tag="oproj" if ncx == 0 else "dummy",
                                name=f"pso_{qc}_{qt}_{ncx}", bufs=1)
                        for pr in range(2):
                            nc.tensor.matmul(
                                ps_op,
                                lhsT=outPs[pr][:, qt * 128:(qt + 1) * 128],
                                rhs=wo_pr[:, pr, ncx * 512:(ncx + 1) * 512],
                                start=(pr == 0), stop=(pr == 1))
                        nc.vector.tensor_copy(
                            osb[:, ncx * 512:(ncx + 1) * 512], ps_op)
                    nc.sync.dma_start(
                        out=out_d.ap()[qc * 512 + qt * 128:
                                       qc * 512 + (qt + 1) * 128, :],
                        in_=osb)

                # ---------------- attention sweep ----------------
                pending_norm = None   # (outP, ps_avs, [uouts], [recips], tag)
                prev_oproj = None     # (qc, outPs) awaiting out_proj
                pair_idx = 0
                for qc in range(4):  # q-chunks of 512
                    qsl = slice(qc * 512, (qc + 1) * 512)
                    outPs = {}
                    # qc3 runs mt1 first so the kernel tail only carries
                    # mt0's normalize
                    for mt in ((1, 0) if qc == 3 else (0, 1)):
                        tag = f"{qc}_{mt}"
                        attnT = at_pool.tile([128, 2, 4, 512], bf16,
                                             tag="at", name=f"at_{tag}")
                        ps_avs = [ps_av_pool.tile([HD, 512], f32, tag="av",
                                                  name=f"av_{tag}_{hh}")
                                  for hh in range(2)]

                        def av_mm(kt, ps_avs=ps_avs, attnT=attnT, mt=mt):
                            for hh in range(2):
                                nc.tensor.matmul(
                                    ps_avs[hh],
                                    lhsT=vp[:, kt, 2 * mt + hh, :],
                                    rhs=attnT[:, hh, kt % 4, :],
                                    start=(kt == 0), stop=(kt == 15))

                        for kt in range(16):
                            # inline QKV fillers, placed just before need
                            if pair_idx == 0:
                                if kt % 4 == 0 and kt > 0:
                                    qk_full(wk_sb, None, kT, kt // 4, 0)
                                if 2 <= kt <= 13:
                                    v_unit(kt + 2)
                            elif pair_idx == 1 and kt % 4 == 0:
                                if kt == 0:
                                    qk_full(wq_sb, bq_sb, qT, 0, 1)
                                qk_full(wk_sb, None, kT, kt // 4, 1)
                            elif pair_idx >= 2 and kt == 0:
                                qk_full(wq_sb, bq_sb, qT, qc, mt)
                            # deferred normalize of the previous pair:
                            # evacuations at kt 0/1 (before av_mm(0) below),
                            # DVE recip chains at kt 2/3, PE bcast+mul at 4/6
                            if pending_norm is not None:
                                pP, pavs, puo, prc, ptag = pending_norm
                                if kt in (0, 1):
                                    puo.append(norm_evac(pavs[kt], kt, ptag))
                                elif kt in (2, 3):
                                    prc.append(norm_recip(puo[kt - 2],
                                                          kt - 2, ptag))
                                elif kt == 4:
                                    norm_fin(pP, puo[0], prc[0], 0, ptag)
                                elif kt == 6:
                                    norm_fin(pP, puo[1], prc[1], 1, ptag)
                                    pending_norm = None
                            # deferred out_proj of the previous q-chunk
                            if prev_oproj is not None and kt in (8, 12):
                                pq, pouts = prev_oproj
                                qt0 = 0 if kt == 8 else 2
                                oproj_unit(pq, pouts, qt0)
                                oproj_unit(pq, pouts, qt0 + 1)
                                if kt == 12:
                                    prev_oproj = None
                            # scores (transposed): S^T[k,q] = kT.T @ qT
                            ps_s = ps_s_pool.tile([128, 2, 512], f32, tag="s")
                            for hh in range(2):
                                nc.tensor.matmul(
                                    ps_s[:, hh, :],
                                    lhsT=kT[hh * 64:hh * 64 + 64, mt,
                                            kt * 128:(kt + 1) * 128],
                                    rhs=qT[hh * 64:hh * 64 + 64, mt, qsl],
                                    start=True, stop=True)
                            nc.scalar.activation(
                                out=attnT[:, :, kt % 4, :], in_=ps_s, func=EXP)
                            if kt >= 2:
                                av_mm(kt - 2)
                        av_mm(14)
                        av_mm(15)

                        outP = op_pool.tile([128, 512], bf16, tag="outP",
                                            name=f"outP_{tag}")
                        pending_norm = (outP, ps_avs, [], [], tag)
                        outPs[mt] = outP
                        pair_idx += 1
                    prev_oproj = (qc, [outPs[0], outPs[1]])

                # tail: last pair's normalize, then final out_proj
                pP, pavs, puo, prc, ptag = pending_norm
                for hh in range(2):
                    puo.append(norm_evac(pavs[hh], hh, ptag))
                for hh in range(2):
                    prc.append(norm_recip(puo[hh], hh, ptag))
                for hh in range(2):
                    norm_fin(pP, puo[hh], prc[hh], hh, ptag)
                pq, pouts = prev_oproj
                for qt in range(4):
                    oproj_unit(pq, pouts, qt, tail=True)

    nc.compile()
    _CACHE["nc"] = nc
    return nc


def make_in_maps(x, Wq, bq, Wk, bk, Wv, bv, Wo):
    import ml_dtypes
    bf = ml_dtypes.bfloat16

    x = np.asarray(x, dtype=np.float32)
    Wq = np.asarray(Wq, dtype=np.float32)
    bq = np.asarray(bq, dtype=np.float32)
    Wk = np.asarray(Wk, dtype=np.float32)
    Wv = np.asarray(Wv, dtype=np.float32)
    bv = np.asarray(bv, dtype=np.float32)
    Wo = np.asarray(Wo, dtype=np.float32)

    scale = np.float32(1.0 / 8.0)  # 1/sqrt(64)

    in_maps = []
    for core in range(N_CORES):
        b = core // 4
        g = core % 4
        cs = slice(g * G, (g + 1) * G)
        in_maps.append({
            "xt": np.ascontiguousarray(
                x[b].reshape(4, 512, 8, 128).transpose(0, 3, 2, 1)).astype(bf),
            "wq": np.ascontiguousarray(Wq[:, cs] * scale).astype(bf),
            "wk": np.ascontiguousarray(Wk[:, cs]).astype(bf),
            "wv": np.ascontiguousarray(Wv[:, cs]).astype(bf),
            "bq": np.ascontiguousarray((bq[cs] * scale).reshape(G, 1)),
            "bv": np.ascontiguousarray(bv[cs]),
            "wo": np.ascontiguousarray(Wo[cs, :].reshape(NHL, 64, H)).astype(bf),
        })
    return in_maps


def kernel(x, Wq, bq, Wk, bk, Wv, bv, Wo, bo):
    from concourse.bass_utils import run_bass_kernel_spmd

    bo = np.asarray(bo, dtype=np.float32)
    nc = _build()
    in_maps = make_in_maps(x, Wq, bq, Wk, bk, Wv, bv, Wo)
    res = run_bass_kernel_spmd(nc, in_maps, core_ids=list(range(N_CORES)))

    out = np.empty((2, S, H), dtype=np.float32)
    for b in range(2):
        acc = res.results[4 * b]["out"].astype(np.float32)
        for g in range(1, 4):
            acc = acc + res.results[4 * b + g]["out"]
        out[b] = acc + bo
    return out


# revision 20
# speedup vs baseline: 1.0801x; 1.0207x over previous
"""Multi-head self-attention TRN2 Bass kernel (v2: PE-saturated schedule).

Problem: x[2, 2048, 1024], 16 heads x 64 dim, fp32.
Sharding: 8 cores = 2 batches x 4 head-groups (4 heads each).
Each core computes its batch's partial output (its 4 heads through
QKV -> attention -> output projection rows); host sums the 4 partials
per batch and adds bo.

The PE (tensor engine) is the bottleneck (~167us of streamed matmul
at bf16: scores 54.6 + attn@V 54.6 + QKV 41 + out_proj 13.6); the
ACT exp stream is ~142us. v2 therefore schedules for zero PE idle:

  - minimal warmup: only kT m=0 chunk 0 (no bias -- bk is dropped
    entirely: it shifts scores by a per-q constant, softmax-invariant),
    v chunks 0-3, qT m=0 chunk 0. Everything else streams into the
    attention sweep as dependency-placed inline fillers.
  - exp activation table pre-loaded during the initial DMA wait;
    dummy matmuls on a memset tile keep the PE p-state warm while
    the first x chunk lands.
  - scores->AV lag of 2 iterations so the in-order PE queue never
    head-stalls on the ACT stream.
  - normalize (evac + reciprocal + rank-1 PE broadcast) of pair p is
    deferred into pair p+1's early iterations; out_proj of q-chunk qc
    runs inside (qc+1, mt=0)'s sweep.
"""

import numpy as np

S = 2048          # sequence length per batch
H = 1024          # hidden
G = 256           # head-group width (4 heads x 64)
HD = 65           # V' columns per head (64 + ones)
NHL = 4           # heads per core
N_CORES = 8

_CACHE = {}


def _build():
    if "nc" in _CACHE:
        return _CACHE["nc"]

    import concourse.bass as bass
    import concourse.mybir as mybir
    import concourse.tile as tile
    from concourse import bacc
    from concourse.tile_rust import add_dep_helper

    f32 = mybir.dt.float32
    bf16 = mybir.dt.bfloat16
    EXP = mybir.ActivationFunctionType.Exp

    nc = bacc.Bacc("TRN2", target_bir_lowering=False, debug=False,
                   num_devices=N_CORES)

    xt_in = nc.dram_tensor("xt", [4, 128, 8, 512], bf16, kind="ExternalInput")
    wq_in = nc.dram_tensor("wq", [H, G], bf16, kind="ExternalInput")
    wk_in = nc.dram_tensor("wk", [H, G], bf16, kind="ExternalInput")
    wv_in = nc.dram_tensor("wv", [H, G], bf16, kind="ExternalInput")
    bq_in = nc.dram_tensor("bq", [G, 1], f32, kind="ExternalInput")
    bv_in = nc.dram_tensor("bv", [G], f32, kind="ExternalInput")
    wo_in = nc.dram_tensor("wo", [NHL, 64, H], bf16, kind="ExternalInput")
    out_d = nc.dram_tensor("out", [S, H], bf16, kind="ExternalOutput")

    with tile.TileContext(nc) as tc:
        with tc.tile_pool(name="persist", bufs=1) as persist:
            qT = persist.tile([128, 2, S], bf16)     # [qd, m, s]
            kT = persist.tile([128, 2, S], bf16)
            vp = persist.tile([128, 16, NHL, HD], bf16)  # [s-part, st, h, col]
            bq_sb = persist.tile([128, 2, 1], f32)
            bv_f = persist.tile([1, G], f32)
            bv_bf = persist.tile([1, G], bf16)
            bv_bc = persist.tile([128, G], f32)
            wo_pr = persist.tile([128, 2, H], bf16)
            ones64 = persist.tile([1, 64], bf16)
            ones128 = persist.tile([1, 128], bf16)
            warm = persist.tile([128, 512], bf16)
            warm_e = persist.tile([1, 8], f32)
            wq_sb = persist.tile([128, 8, G], bf16)
            wk_sb = persist.tile([128, 8, G], bf16)
            wv_sb = persist.tile([128, 8, G], bf16)
            xTc = [persist.tile([128, 8, 512], bf16, name=f"xT_{jc}")
                   for jc in range(4)]

            with (
                tc.tile_pool(name="at_roll", bufs=2) as at_pool,
                tc.tile_pool(name="outP", bufs=4) as op_pool,
                tc.tile_pool(name="tmpo", bufs=1) as tmpo_pool,
                tc.tile_pool(name="sums", bufs=4) as sums_pool,
                tc.tile_pool(name="osb", bufs=2) as osb_pool,
                tc.tile_pool(name="ps_s", bufs=2, space="PSUM") as ps_s_pool,
                tc.tile_pool(name="ps_av", bufs=2, space="PSUM") as ps_av_pool,
                tc.tile_pool(name="ps_op", bufs=1, space="PSUM") as ps_op_pool,
            ):
                # memsets first so the gpsimd queue isn't blocked
                nc.gpsimd.memset(warm, 0.125)
                nc.gpsimd.memset(vp[:, :, :, 64:65], 1.0)
                nc.gpsimd.memset(ones64, 1.0)
                nc.gpsimd.memset(ones128, 1.0)

                # ---------------- DMAs (two priority chains) ----------------
                # chain A (small): bv -> wk -> wv -> wq -> bq
                dma_bv = nc.sync.dma_start(
                    out=bv_f, in_=bv_in.ap().rearrange("(o g) -> o g", o=1))
                dma_wk = nc.sync.dma_start(
                    out=wk_sb, in_=wk_in.ap().rearrange("(t p) d -> p t d", p=128))
                dma_wv = nc.sync.dma_start(
                    out=wv_sb, in_=wv_in.ap().rearrange("(t p) d -> p t d", p=128))
                dma_wq = nc.sync.dma_start(
                    out=wq_sb, in_=wq_in.ap().rearrange("(t p) d -> p t d", p=128))
                dma_bq = nc.sync.dma_start(
                    out=bq_sb, in_=bq_in.ap().rearrange("(m p) o -> p m o", p=128))
                # chain B (big): xc0 (split in halves so the warmup matmuls
                # can start after 512KB) -> xc1 -> xc2 -> xc3 -> wo
                x0a = nc.sync.dma_start(out=xTc[0][:, 0:4, :],
                                        in_=xt_in.ap()[0][:, 0:4, :])
                x0b = nc.sync.dma_start(out=xTc[0][:, 4:8, :],
                                        in_=xt_in.ap()[0][:, 4:8, :])
                x_dmas = [x0b] + [nc.sync.dma_start(out=xTc[jc],
                                                    in_=xt_in.ap()[jc])
                          for jc in range(1, 4)]
                add_dep_helper(x0b.ins, x0a.ins, reason="dma order")
                # Wo as stacked head pairs: [two*64+p, pr, n]
                dma_wo = nc.sync.dma_start(
                    out=wo_pr,
                    in_=wo_in.ap().rearrange("(pr two) p n -> (two p) pr n", two=2))
                for a, b in [(dma_wk, dma_bv), (dma_wv, dma_wk),
                             (dma_wq, dma_wv), (dma_bq, dma_wq),
                             (x_dmas[1], x_dmas[0]), (x_dmas[2], x_dmas[1]),
                             (x_dmas[3], x_dmas[2]), (dma_wo, x_dmas[3])]:
                    add_dep_helper(a.ins, b.ins, reason="dma order")

                # pre-load the exp activation table (~2.7us) off the
                # critical path
                nc.scalar.activation(out=warm_e, in_=warm[0:1, 0:8], func=EXP)

                def dummy(n=512):
                    ps_d = ps_op_pool.tile([128, 512], f32, tag="dummy",
                                           bufs=1)
                    nc.tensor.matmul(ps_d[:, 0:n], lhsT=warm[:, 0:128],
                                     rhs=warm[:, 0:n], start=True, stop=True)

                for _ in range(4):
                    dummy()
                # bv broadcast along partitions via rank-1 PE outer product
                nc.vector.tensor_copy(bv_bf, bv_f)
                ps_bv = ps_op_pool.tile([128, G], f32, tag="oproj",
                                        name="ps_bv", bufs=1)
                nc.tensor.matmul(ps_bv, lhsT=ones128, rhs=bv_bf,
                                 start=True, stop=True)
                nc.vector.tensor_copy(bv_bc, ps_bv)
                for _ in range(4):
                    dummy()

                # ---------------- QKV building blocks ----------------
                qk_ring = [0]

                def qk_full(w_sb, b_sb, dst, jc, m):
                    """dst[:, m, jc-chunk] = (x @ W)[:, m-half] (+ bias).

                    PSUM comes from the two 1-buf rings (dummy/oproj)
                    alternately so adjacent calls don't WAR-stall."""
                    sl = slice(jc * 512, (jc + 1) * 512)
                    tag = "dummy" if qk_ring[0] == 0 else "oproj"
                    qk_ring[0] ^= 1
                    ps = ps_op_pool.tile([128, 512], f32, tag=tag,
                                         name=f"psqk_{id(w_sb)}_{jc}_{m}",
                                         bufs=1)
                    for ht in range(8):
                        nc.tensor.matmul(
                            ps,
                            lhsT=w_sb[:, ht, m * 128:(m + 1) * 128],
                            rhs=xTc[jc][:, ht, :],
                            start=(ht == 0), stop=(ht == 7))
                    if b_sb is not None:
                        nc.vector.tensor_scalar_add(dst[:, m, sl], ps,
                                                    b_sb[:, m, :])
                    else:
                        nc.vector.tensor_copy(dst[:, m, sl], ps)

                def v_unit(st16):
                    tag = "dummy" if qk_ring[0] == 0 else "oproj"
                    qk_ring[0] ^= 1
                    ps_vt = ps_op_pool.tile([128, 512], f32, tag=tag,
                                            name=f"psv_{st16}", bufs=1)
                    for ht in range(8):
                        nc.tensor.matmul(
                            ps_vt[:, 0:G],
                            lhsT=xTc[st16 // 4][:, ht,
                                                (st16 % 4) * 128:
                                                (st16 % 4 + 1) * 128],
                            rhs=wv_sb[:, ht, :],
                            start=(ht == 0), stop=(ht == 7))
                    nc.vector.tensor_add(
                        vp[:, st16, :, 0:64],
                        ps_vt[:, 0:G].rearrange("p (h d) -> p h d", h=NHL),
                        bv_bc.rearrange("p (h d) -> p h d", h=NHL))

                # warmup: exactly what pair 0 iterations 0-3 need
                qk_full(wk_sb, None, kT, 0, 0)
                for i in range(4):
                    v_unit(i)
                qk_full(wq_sb, bq_sb, qT, 0, 0)

                # ---------------- normalize + out_proj ----------------
                def norm_evac(ps_av, hh, tag):
                    # evacuate PSUM right away to release the bank; MUST be
                    # emitted before the next pair's first AV matmul so the
                    # ring WAR dependency is seen
                    uout = tmpo_pool.tile([HD, 512], f32, tag="uout",
                                          name=f"uo_{tag}_{hh}", bufs=4)
                    nc.vector.tensor_copy(uout, ps_av)
                    return uout

                def norm_recip(uout, hh, tag):
                    # DVE chain: sums copy -> fast reciprocal -> bf16 cast
                    sums = sums_pool.tile([1, 512], f32, tag="sums",
                                          name=f"sm_{tag}_{hh}")
                    nc.vector.tensor_copy(sums, uout[64:65, :])
                    recip = sums_pool.tile([1, 512], f32, tag="recip",
                                           name=f"rc_{tag}_{hh}")
                    nc.vector.reciprocal_approx_fast(out=recip, in_=sums)
                    recip_bf = sums_pool.tile([1, 512], bf16, tag="recipb",
                                              name=f"rcb_{tag}_{hh}")
                    nc.vector.tensor_copy(recip_bf, recip)
                    return recip_bf

                def norm_fin(outP, uout, recip_bf, hh, tag):
                    # broadcast along partitions on the idle GPSIMD engine
                    # (keeps the PE out of the normalize chain entirely)
                    rbc = sums_pool.tile([64, 512], bf16, tag="rbc",
                                         name=f"rb_{tag}_{hh}")
                    nc.gpsimd.partition_broadcast(rbc, recip_bf)
                    nc.vector.tensor_mul(
                        outP[hh * 64:hh * 64 + 64, :], uout[0:64, :], rbc)

                def oproj_unit(qc, outPs, qt, tail=False):
                    # out_proj for one q-tile (K=128 stacked pairs); the two
                    # ncx halves alternate the 1-buf rings so ncx1's matmuls
                    # don't WAR-stall on ncx0's evacuation; at the kernel
                    # tail the freed score slots double-buffer it
                    osb = osb_pool.tile([128, H], bf16, tag="osb",
                                        name=f"osb_{qc}_{qt}")
                    for ncx in range(2):
                        if tail:
                            ps_op = ps_s_pool.tile(
                                [128, 2, 512], f32, tag="s",
                                name=f"psot_{qc}_{qt}_{ncx}")[:, 0, :]
                        else:
                            ps_op = ps_op_pool.tile(
                                [128, 512], f32,
                                tag="oproj" if ncx == 0 else "dummy",
                                name=f"pso_{qc}_{qt}_{ncx}", bufs=1)
                        for pr in range(2):
                            nc.tensor.matmul(
                                ps_op,
                                lhsT=outPs[pr][:, qt * 128:(qt + 1) * 128],
                                rhs=wo_pr[:, pr, ncx * 512:(ncx + 1) * 512],
                                start=(pr == 0), stop=(pr == 1))
                        if tail and ncx == 1:
                            # ACT is idle after the last exp -- split the
                            # tail evacuations across ACT and DVE
                            nc.scalar.copy(
                                osb[:, ncx * 512:(ncx + 1) * 512], ps_op)
                        else:
                            nc.vector.tensor_copy(
                                osb[:, ncx * 512:(ncx + 1) * 512], ps_op)
                    nc.sync.dma_start(
                        out=out_d.ap()[qc * 512 + qt * 128:
                                       qc * 512 + (qt + 1) * 128, :],
                        in_=osb)

                # ---------------- attention sweep ----------------
                pending_norm = None   # (outP, ps_avs, [uouts], [recips], tag)
                prev_oproj = None     # (qc, outPs) awaiting out_proj
                pair_idx = 0
                for qc in range(4):  # q-chunks of 512
                    qsl = slice(qc * 512, (qc + 1) * 512)
                    outPs = {}
                    # qc3 runs mt1 first so the kernel tail only carries
                    # mt0's normalize
                    for mt in ((1, 0) if qc == 3 else (0, 1)):
                        tag = f"{qc}_{mt}"
                        attnT = at_pool.tile([128, 2, 4, 512], bf16,
                                             tag="at", name=f"at_{tag}")
                        ps_avs = [ps_av_pool.tile([HD, 512], f32, tag="av",
                                                  name=f"av_{tag}_{hh}")
                                  for hh in range(2)]

                        def av_mm(kt, ps_avs=ps_avs, attnT=attnT, mt=mt):
                            for hh in range(2):
                                nc.tensor.matmul(
                                    ps_avs[hh],
                                    lhsT=vp[:, kt, 2 * mt + hh, :],
                                    rhs=attnT[:, hh, kt % 4, :],
                                    start=(kt == 0), stop=(kt == 15))

                        for kt in range(16):
                            # inline QKV fillers, placed just before need
                            if pair_idx == 0:
                                if kt % 4 == 0 and kt > 0:
                                    qk_full(wk_sb, None, kT, kt // 4, 0)
                                if 2 <= kt <= 13:
                                    v_unit(kt + 2)
                            elif pair_idx == 1 and kt % 4 == 0:
                                if kt == 0:
                                    qk_full(wq_sb, bq_sb, qT, 0, 1)
                                qk_full(wk_sb, None, kT, kt // 4, 1)
                            elif pair_idx >= 2 and kt == 0:
                                qk_full(wq_sb, bq_sb, qT, qc, mt)
                            # deferred normalize of the previous pair:
                            # evacuations at kt 0/1 (before av_mm(0) below),
                            # DVE recip chains at kt 2/3, PE bcast+mul at 4/6
                            if pending_norm is not None:
                                pP, pavs, puo, prc, ptag = pending_norm
                                if kt in (0, 1):
                                    puo.append(norm_evac(pavs[kt], kt, ptag))
                                elif kt in (2, 3):
                                    prc.append(norm_recip(puo[kt - 2],
                                                          kt - 2, ptag))
                                elif kt == 4:
                                    norm_fin(pP, puo[0], prc[0], 0, ptag)
                                elif kt == 6:
                                    norm_fin(pP, puo[1], prc[1], 1, ptag)
                                    pending_norm = None
                            # deferred out_proj of the previous q-chunk
                            if prev_oproj is not None and kt in (8, 12):
                                pq, pouts = prev_oproj
                                qt0 = 0 if kt == 8 else 2
                                oproj_unit(pq, pouts, qt0)
                                oproj_unit(pq, pouts, qt0 + 1)
                                if kt == 12:
                                    prev_oproj = None
                            # scores (transposed): S^T[k,q] = kT.T @ qT
                            ps_s = ps_s_pool.tile([128, 2, 512], f32, tag="s")
                            for hh in range(2):
                                nc.tensor.matmul(
                                    ps_s[:, hh, :],
                                    lhsT=kT[hh * 64:hh * 64 + 64, mt,
                                            kt * 128:(kt + 1) * 128],
                                    rhs=qT[hh * 64:hh * 64 + 64, mt, qsl],
                                    start=True, stop=True)
                            nc.scalar.activation(
                                out=attnT[:, :, kt % 4, :], in_=ps_s, func=EXP)
                            if kt >= 2:
                                av_mm(kt - 2)
                        av_mm(14)
                        av_mm(15)

                        outP = op_pool.tile([128, 512], bf16, tag="outP",
                                            name=f"outP_{tag}")
                        pending_norm = (outP, ps_avs, [], [], tag)
                        outPs[mt] = outP
                        pair_idx += 1
                    prev_oproj = (qc, [outPs[0], outPs[1]])

                # tail: last pair's normalize, then final out_proj
                pP, pavs, puo, prc, ptag = pending_norm
                for hh in range(2):
                    puo.append(norm_evac(pavs[hh], hh, ptag))
                for hh in range(2):
                    prc.append(norm_recip(puo[hh], hh, ptag))
                for hh in range(2):
                    norm_fin(pP, puo[hh], prc[hh], hh, ptag)
                pq, pouts = prev_oproj
                for qt in range(4):
                    oproj_unit(pq, pouts, qt, tail=True)

    nc.compile()
    _CACHE["nc"] = nc
    return nc


def make_in_maps(x, Wq, bq, Wk, bk, Wv, bv, Wo):
    import ml_dtypes
    bf = ml_dtypes.bfloat16

    x = np.asarray(x, dtype=np.float32)
    Wq = np.asarray(Wq, dtype=np.float32)
    bq = np.asarray(bq, dtype=np.float32)
    Wk = np.asarray(Wk, dtype=np.float32)
    Wv = np.asarray(Wv, dtype=np.float32)
    bv = np.asarray(bv, dtype=np.float32)
    Wo = np.asarray(Wo, dtype=np.float32)

    scale = np.float32(1.0 / 8.0)  # 1/sqrt(64)

    in_maps = []
    for core in range(N_CORES):
        b = core // 4
        g = core % 4
        cs = slice(g * G, (g + 1) * G)
        in_maps.append({
            "xt": np.ascontiguousarray(
                x[b].reshape(4, 512, 8, 128).transpose(0, 3, 2, 1)).astype(bf),
            "wq": np.ascontiguousarray(Wq[:, cs] * scale).astype(bf),
            "wk": np.ascontiguousarray(Wk[:, cs]).astype(bf),
            "wv": np.ascontiguousarray(Wv[:, cs]).astype(bf),
            "bq": np.ascontiguousarray((bq[cs] * scale).reshape(G, 1)),
            "bv": np.ascontiguousarray(bv[cs]),
            "wo": np.ascontiguousarray(Wo[cs, :].reshape(NHL, 64, H)).astype(bf),
        })
    return in_maps


def kernel(x, Wq, bq, Wk, bk, Wv, bv, Wo, bo):
    from concourse.bass_utils import run_bass_kernel_spmd

    bo = np.asarray(bo, dtype=np.float32)
    nc = _build()
    in_maps = make_in_maps(x, Wq, bq, Wk, bk, Wv, bv, Wo)
    res = run_bass_kernel_spmd(nc, in_maps, core_ids=list(range(N_CORES)))

    out = np.empty((2, S, H), dtype=np.float32)
    for b in range(2):
        acc = res.results[4 * b]["out"].astype(np.float32)
        for g in range(1, 4):
            acc = acc + res.results[4 * b + g]["out"]
        out[b] = acc + bo
    return out


# revision 27
# speedup vs baseline: 1.0929x; 1.0118x over previous
"""Multi-head self-attention TRN2 Bass kernel (v2: PE-saturated schedule).

Problem: x[2, 2048, 1024], 16 heads x 64 dim, fp32.
Sharding: 8 cores = 2 batches x 4 head-groups (4 heads each).
Each core computes its batch's partial output (its 4 heads through
QKV -> attention -> output projection rows); host sums the 4 partials
per batch and adds bo.

The PE (tensor engine) is the bottleneck (~167us of streamed matmul
at bf16: scores 54.6 + attn@V 54.6 + QKV 41 + out_proj 13.6); the
ACT exp stream is ~142us. v2 therefore schedules for zero PE idle:

  - minimal warmup: only kT m=0 chunk 0 (no bias -- bk is dropped
    entirely: it shifts scores by a per-q constant, softmax-invariant),
    v chunks 0-3, qT m=0 chunk 0. Everything else streams into the
    attention sweep as dependency-placed inline fillers.
  - exp activation table pre-loaded during the initial DMA wait;
    dummy matmuls on a memset tile keep the PE p-state warm while
    the first x chunk lands.
  - scores->AV lag of 2 iterations so the in-order PE queue never
    head-stalls on the ACT stream.
  - normalize (evac + reciprocal + rank-1 PE broadcast) of pair p is
    deferred into pair p+1's early iterations; out_proj of q-chunk qc
    runs inside (qc+1, mt=0)'s sweep.
"""

import numpy as np

S = 2048          # sequence length per batch
H = 1024          # hidden
G = 256           # head-group width (4 heads x 64)
HD = 65           # V' columns per head (64 + ones)
NHL = 4           # heads per core
N_CORES = 8

_CACHE = {}


def _build():
    if "nc" in _CACHE:
        return _CACHE["nc"]

    import concourse.bass as bass
    import concourse.mybir as mybir
    import concourse.tile as tile
    from concourse import bacc
    from concourse.tile_rust import add_dep_helper

    f32 = mybir.dt.float32
    bf16 = mybir.dt.bfloat16
    f8 = mybir.dt.float8e4
    EXP = mybir.ActivationFunctionType.Exp
    DR = mybir.MatmulPerfMode.DoubleRow

    nc = bacc.Bacc("TRN2", target_bir_lowering=False, debug=False,
                   num_devices=N_CORES)

    xt_in = nc.dram_tensor("xt", [4, 128, 8, 512], bf16, kind="ExternalInput")
    wq_in = nc.dram_tensor("wq", [H, G], bf16, kind="ExternalInput")
    wk_in = nc.dram_tensor("wk", [H, G], bf16, kind="ExternalInput")
    wv_in = nc.dram_tensor("wv", [H, G], bf16, kind="ExternalInput")
    bq_in = nc.dram_tensor("bq", [G, 1], f32, kind="ExternalInput")
    bv_in = nc.dram_tensor("bv", [G], f32, kind="ExternalInput")
    wo_in = nc.dram_tensor("wo", [NHL, 64, H], bf16, kind="ExternalInput")
    out_d = nc.dram_tensor("out", [S, H], bf16, kind="ExternalOutput")

    with tile.TileContext(nc) as tc:
        with tc.tile_pool(name="persist", bufs=1) as persist:
            qT = persist.tile([128, 2, S], bf16)     # [qd, m, s]
            kT = persist.tile([128, 2, S], bf16)
            vp = persist.tile([128, 16, NHL, HD], bf16)  # [s-part, st, h, col]
            bq_sb = persist.tile([128, 2, 1], f32)
            bv_f = persist.tile([1, G], f32)
            bv_bf = persist.tile([1, G], bf16)
            bv_bc = persist.tile([128, G], f32)
            wo_pr = persist.tile([128, 2, H], bf16)
            ones64 = persist.tile([1, 64], bf16)
            ones128 = persist.tile([1, 128], bf16)
            warm = persist.tile([128, 512], bf16)
            warm_e = persist.tile([1, 8], f32)
            wq_sb = persist.tile([128, 8, G], bf16)
            wk_sb = persist.tile([128, 8, G], bf16)
            wv_sb = persist.tile([128, 8, G], bf16)
            xTc = [persist.tile([128, 8, 512], bf16, name=f"xT_{jc}")
                   for jc in range(4)]

            with (
                tc.tile_pool(name="at_roll", bufs=2) as at_pool,
                tc.tile_pool(name="outP", bufs=4) as op_pool,
                tc.tile_pool(name="tmpo", bufs=1) as tmpo_pool,
                tc.tile_pool(name="sums", bufs=4) as sums_pool,
                tc.tile_pool(name="osb", bufs=2) as osb_pool,
                tc.tile_pool(name="ps_s", bufs=2, space="PSUM") as ps_s_pool,
                tc.tile_pool(name="ps_av", bufs=2, space="PSUM") as ps_av_pool,
                tc.tile_pool(name="ps_op", bufs=1, space="PSUM") as ps_op_pool,
            ):
                # memsets first so the gpsimd queue isn't blocked
                nc.gpsimd.memset(warm, 0.125)
                nc.gpsimd.memset(vp[:, :, :, 64:65], 1.0)
                nc.gpsimd.memset(ones64, 1.0)
                nc.gpsimd.memset(ones128, 1.0)

                # ---------------- DMAs (two priority chains) ----------------
                # chain A (small): bv -> wk -> wv -> wq -> bq
                dma_bv = nc.sync.dma_start(
                    out=bv_f, in_=bv_in.ap().rearrange("(o g) -> o g", o=1))
                dma_wk = nc.sync.dma_start(
                    out=wk_sb, in_=wk_in.ap().rearrange("(t p) d -> p t d", p=128))
                dma_wv = nc.sync.dma_start(
                    out=wv_sb, in_=wv_in.ap().rearrange("(t p) d -> p t d", p=128))
                dma_wq = nc.sync.dma_start(
                    out=wq_sb, in_=wq_in.ap().rearrange("(t p) d -> p t d", p=128))
                dma_bq = nc.sync.dma_start(
                    out=bq_sb, in_=bq_in.ap().rearrange("(m p) o -> p m o", p=128))
                # chain B (big): xc0 (split in halves so the warmup matmuls
                # can start after 512KB) -> xc1 -> xc2 -> xc3 -> wo
                x0a = nc.sync.dma_start(out=xTc[0][:, 0:4, :],
                                        in_=xt_in.ap()[0][:, 0:4, :])
                x0b = nc.sync.dma_start(out=xTc[0][:, 4:8, :],
                                        in_=xt_in.ap()[0][:, 4:8, :])
                x_dmas = [x0b] + [nc.sync.dma_start(out=xTc[jc],
                                                    in_=xt_in.ap()[jc])
                          for jc in range(1, 4)]
                # Wo as stacked head pairs: [two*64+p, pr, n]
                dma_wo = nc.sync.dma_start(
                    out=wo_pr,
                    in_=wo_in.ap().rearrange("(pr two) p n -> (two p) pr n", two=2))
                add_dep_helper(x0b.ins, x0a.ins, reason="dma order")
                for a, b in [(dma_wk, dma_bv), (dma_wv, dma_wk),
                             (dma_wq, dma_wv), (dma_bq, dma_wq),
                             (x_dmas[1], x_dmas[0]), (x_dmas[2], x_dmas[1]),
                             (x_dmas[3], x_dmas[2]), (dma_wo, x_dmas[3])]:
                    add_dep_helper(a.ins, b.ins, reason="dma order")

                # pre-load the exp activation table (~2.7us) off the
                # critical path
                nc.scalar.activation(out=warm_e, in_=warm[0:1, 0:8], func=EXP)

                def dummy(n=512):
                    ps_d = ps_op_pool.tile([128, 512], f32, tag="dummy",
                                           bufs=1)
                    nc.tensor.matmul(ps_d[:, 0:n], lhsT=warm[:, 0:128],
                                     rhs=warm[:, 0:n], start=True, stop=True)

                for _ in range(4):
                    dummy()
                # bv broadcast along partitions via rank-1 PE outer product
                nc.vector.tensor_copy(bv_bf, bv_f)
                ps_bv = ps_op_pool.tile([128, G], f32, tag="oproj",
                                        name="ps_bv", bufs=1)
                nc.tensor.matmul(ps_bv, lhsT=ones128, rhs=bv_bf,
                                 start=True, stop=True)
                nc.vector.tensor_copy(bv_bc, ps_bv)
                for _ in range(4):
                    dummy()

                # ---------------- QKV building blocks ----------------
                qk_ring = [0]

                def qk_full(w_sb, b_sb, dst, jc, m):
                    """dst[:, m, jc-chunk] = (x @ W)[:, m-half] (+ bias).

                    PSUM comes from the two 1-buf rings (dummy/oproj)
                    alternately so adjacent calls don't WAR-stall."""
                    sl = slice(jc * 512, (jc + 1) * 512)
                    tag = "dummy" if qk_ring[0] == 0 else "oproj"
                    qk_ring[0] ^= 1
                    ps = ps_op_pool.tile([128, 512], f32, tag=tag,
                                         name=f"psqk_{id(w_sb)}_{jc}_{m}",
                                         bufs=1)
                    for ht in range(8):
                        nc.tensor.matmul(
                            ps,
                            lhsT=w_sb[:, ht, m * 128:(m + 1) * 128],
                            rhs=xTc[jc][:, ht, :],
                            start=(ht == 0), stop=(ht == 7))
                    if b_sb is not None:
                        nc.vector.tensor_scalar_add(dst[:, m, sl], ps,
                                                    b_sb[:, m, :])
                    else:
                        nc.vector.tensor_copy(dst[:, m, sl], ps)

                def v_unit(st16):
                    tag = "dummy" if qk_ring[0] == 0 else "oproj"
                    qk_ring[0] ^= 1
                    ps_vt = ps_op_pool.tile([128, 512], f32, tag=tag,
                                            name=f"psv_{st16}", bufs=1)
                    for ht in range(8):
                        nc.tensor.matmul(
                            ps_vt[:, 0:G],
                            lhsT=xTc[st16 // 4][:, ht,
                                                (st16 % 4) * 128:
                                                (st16 % 4 + 1) * 128],
                            rhs=wv_sb[:, ht, :],
                            start=(ht == 0), stop=(ht == 7))
                    nc.vector.tensor_add(
                        vp[:, st16, :, 0:64],
                        ps_vt[:, 0:G].rearrange("p (h d) -> p h d", h=NHL),
                        bv_bc.rearrange("p (h d) -> p h d", h=NHL))

                # warmup: exactly what pair 0 iterations 0-3 need
                qk_full(wk_sb, None, kT, 0, 0)
                for i in range(4):
                    v_unit(i)
                qk_full(wq_sb, bq_sb, qT, 0, 0)

                # ---------------- normalize + out_proj ----------------
                def norm_evac(ps_av, hh, tag):
                    # evacuate PSUM right away to release the bank; MUST be
                    # emitted before the next pair's first AV matmul so the
                    # ring WAR dependency is seen
                    uout = tmpo_pool.tile([HD, 512], f32, tag="uout",
                                          name=f"uo_{tag}_{hh}", bufs=4)
                    nc.vector.tensor_copy(uout, ps_av)
                    return uout

                def norm_recip(uout, hh, tag):
                    # DVE chain: sums copy -> fast reciprocal -> bf16 cast
                    sums = sums_pool.tile([1, 512], f32, tag="sums",
                                          name=f"sm_{tag}_{hh}")
                    nc.vector.tensor_copy(sums, uout[64:65, :])
                    recip = sums_pool.tile([1, 512], f32, tag="recip",
                                           name=f"rc_{tag}_{hh}")
                    nc.vector.reciprocal_approx_fast(out=recip, in_=sums)
                    recip_bf = sums_pool.tile([1, 512], bf16, tag="recipb",
                                              name=f"rcb_{tag}_{hh}")
                    nc.vector.tensor_copy(recip_bf, recip)
                    return recip_bf

                def norm_fin(outP, uout, recip_bf, hh, tag):
                    # broadcast along partitions on the idle GPSIMD engine
                    # (keeps the PE out of the normalize chain entirely)
                    rbc = sums_pool.tile([64, 512], bf16, tag="rbc",
                                         name=f"rb_{tag}_{hh}")
                    nc.gpsimd.partition_broadcast(rbc, recip_bf)
                    nc.vector.tensor_mul(
                        outP[hh * 64:hh * 64 + 64, :], uout[0:64, :], rbc)

                def oproj_unit(qc, outPs, qt, tail=False):
                    # out_proj for one q-tile (K=128 stacked pairs); the two
                    # ncx halves alternate the 1-buf rings so ncx1's matmuls
                    # don't WAR-stall on ncx0's evacuation; at the kernel
                    # tail the freed score slots double-buffer it
                    osb = osb_pool.tile([128, H], bf16, tag="osb",
                                        name=f"osb_{qc}_{qt}")
                    for ncx in range(2):
                        if tail:
                            ps_op = ps_s_pool.tile(
                                [128, 2, 512], f32, tag="s",
                                name=f"psot_{qc}_{qt}_{ncx}")[:, 0, :]
                        else:
                            ps_op = ps_op_pool.tile(
                                [128, 512], f32,
                                tag="oproj" if ncx == 0 else "dummy",
                                name=f"pso_{qc}_{qt}_{ncx}", bufs=1)
                        for pr in range(2):
                            nc.tensor.matmul(
                                ps_op,
                                lhsT=outPs[pr][:, qt * 128:(qt + 1) * 128],
                                rhs=wo_pr[:, pr, ncx * 512:(ncx + 1) * 512],
                                start=(pr == 0), stop=(pr == 1))
                        if tail and ncx == 1:
                            # ACT is idle after the last exp -- split the
                            # tail evacuations across ACT and DVE
                            nc.scalar.copy(
                                osb[:, ncx * 512:(ncx + 1) * 512], ps_op)
                        else:
                            nc.vector.tensor_copy(
                                osb[:, ncx * 512:(ncx + 1) * 512], ps_op)
                    nc.sync.dma_start(
                        out=out_d.ap()[qc * 512 + qt * 128:
                                       qc * 512 + (qt + 1) * 128, :],
                        in_=osb)

                # ---------------- attention sweep ----------------
                pending_norm = None   # (outP, ps_avs, [uouts], [recips], tag)
                prev_oproj = None     # (qc, outPs) awaiting out_proj
                pair_idx = 0
                for qc in range(4):  # q-chunks of 512
                    qsl = slice(qc * 512, (qc + 1) * 512)
                    outPs = {}
                    # qc3 runs mt1 first so the kernel tail only carries
                    # mt0's normalize
                    for mt in ((1, 0) if qc == 3 else (0, 1)):
                        tag = f"{qc}_{mt}"
                        attnT = at_pool.tile([128, 2, 4, 512], bf16,
                                             tag="at", name=f"at_{tag}")
                        ps_avs = [ps_av_pool.tile([HD, 512], f32, tag="av",
                                                  name=f"av_{tag}_{hh}")
                                  for hh in range(2)]

                        def av_mm(kt, ps_avs=ps_avs, attnT=attnT, mt=mt):
                            for hh in range(2):
                                nc.tensor.matmul(
                                    ps_avs[hh],
                                    lhsT=vp[:, kt, 2 * mt + hh, :],
                                    rhs=attnT[:, hh, kt % 4, :],
                                    start=(kt == 0), stop=(kt == 15))

                        for kt in range(16):
                            # inline QKV fillers, placed just before need
                            if pair_idx == 0:
                                if kt % 4 == 0 and kt > 0:
                                    qk_full(wk_sb, None, kT, kt // 4, 0)
                                if 2 <= kt <= 13:
                                    v_unit(kt + 2)
                            elif pair_idx == 1 and kt % 4 == 0:
                                if kt == 0:
                                    qk_full(wq_sb, bq_sb, qT, 0, 1)
                                qk_full(wk_sb, None, kT, kt // 4, 1)
                            elif pair_idx >= 2 and kt == 0:
                                qk_full(wq_sb, bq_sb, qT, qc, mt)
                            # deferred normalize of the previous pair:
                            # evacuations at kt 0/1 (before av_mm(0) below),
                            # DVE recip chains at kt 2/3, PE bcast+mul at 4/6
                            if pending_norm is not None:
                                pP, pavs, puo, prc, ptag = pending_norm
                                if kt in (0, 1):
                                    puo.append(norm_evac(pavs[kt], kt, ptag))
                                elif kt in (2, 3):
                                    prc.append(norm_recip(puo[kt - 2],
                                                          kt - 2, ptag))
                                elif kt == 4:
                                    norm_fin(pP, puo[0], prc[0], 0, ptag)
                                elif kt == 6:
                                    norm_fin(pP, puo[1], prc[1], 1, ptag)
                                    pending_norm = None
                            # deferred out_proj of the previous q-chunk
                            if prev_oproj is not None and kt in (8, 12):
                                pq, pouts = prev_oproj
                                qt0 = 0 if kt == 8 else 2
                                oproj_unit(pq, pouts, qt0)
                                oproj_unit(pq, pouts, qt0 + 1)
                                if kt == 12:
                                    prev_oproj = None
                            # scores (transposed): S^T[k,q] = kT.T @ qT
                            ps_s = ps_s_pool.tile([128, 2, 512], f32, tag="s")
                            for hh in range(2):
                                nc.tensor.matmul(
                                    ps_s[:, hh, :],
                                    lhsT=kT[hh * 64:hh * 64 + 64, mt,
                                            kt * 128:(kt + 1) * 128],
                                    rhs=qT[hh * 64:hh * 64 + 64, mt, qsl],
                                    start=True, stop=True)
                            nc.scalar.activation(
                                out=attnT[:, :, kt % 4, :], in_=ps_s, func=EXP)
                            if kt >= 2:
                                av_mm(kt - 2)
                        av_mm(14)
                        av_mm(15)

                        outP = op_pool.tile([128, 512], bf16, tag="outP",
                                            name=f"outP_{tag}")
                        pending_norm = (outP, ps_avs, [], [], tag)
                        outPs[mt] = outP
                        pair_idx += 1
                    prev_oproj = (qc, [outPs[0], outPs[1]])

                # tail: last pair's normalize, then final out_proj
                pP, pavs, puo, prc, ptag = pending_norm
                for hh in range(2):
                    puo.append(norm_evac(pavs[hh], hh, ptag))
                for hh in range(2):
                    prc.append(norm_recip(puo[hh], hh, ptag))
                for hh in range(2):
                    norm_fin(pP, puo[hh], prc[hh], hh, ptag)
                pq, pouts = prev_oproj
                for qt in range(4):
                    oproj_unit(pq, pouts, qt, tail=True)

    nc.compile()
    _CACHE["nc"] = nc
    return nc


def make_in_maps(x, Wq, bq, Wk, bk, Wv, bv, Wo):
    import ml_dtypes
    bf = ml_dtypes.bfloat16

    x = np.asarray(x, dtype=np.float32)
    Wq = np.asarray(Wq, dtype=np.float32)
    bq = np.asarray(bq, dtype=np.float32)
    Wk = np.asarray(Wk, dtype=np.float32)
    Wv = np.asarray(Wv, dtype=np.float32)
    bv = np.asarray(bv, dtype=np.float32)
    Wo = np.asarray(Wo, dtype=np.float32)

    scale = np.float32(1.0 / 8.0)  # 1/sqrt(64)

    in_maps = []
    for core in range(N_CORES):
        b = core // 4
        g = core % 4
        cs = slice(g * G, (g + 1) * G)
        in_maps.append({
            "xt": np.ascontiguousarray(
                x[b].reshape(4, 512, 8, 128).transpose(0, 3, 2, 1)).astype(bf),
            "wq": np.ascontiguousarray(Wq[:, cs] * scale).astype(bf),
            "wk": np.ascontiguousarray(Wk[:, cs]).astype(bf),
            "wv": np.ascontiguousarray(Wv[:, cs]).astype(bf),
            "bq": np.ascontiguousarray((bq[cs] * scale).reshape(G, 1)),
            "bv": np.ascontiguousarray(bv[cs]),
            "wo": np.ascontiguousarray(Wo[cs, :].reshape(NHL, 64, H)).astype(bf),
        })
    return in_maps


def kernel(x, Wq, bq, Wk, bk, Wv, bv, Wo, bo):
    from concourse.bass_utils import run_bass_kernel_spmd

    bo = np.asarray(bo, dtype=np.float32)
    nc = _build()
    in_maps = make_in_maps(x, Wq, bq, Wk, bk, Wv, bv, Wo)
    res = run_bass_kernel_spmd(nc, in_maps, core_ids=list(range(N_CORES)))

    out = np.empty((2, S, H), dtype=np.float32)
    for b in range(2):
        acc = res.results[4 * b]["out"].astype(np.float32)
        for g in range(1, 4):
            acc = acc + res.results[4 * b + g]["out"]
        out[b] = acc + bo
    return out
